# revision 96
# baseline (speedup 1.0000x reference)
"""GNN (GENConv x2 + TopK pool) Bass/Tile kernel for TRN2, data-parallel over
8 NeuronCores (8 graphs per core).

Edge aggregation uses a fixed-degree main layout: node n owns edge slots
[32n, 32n+32) (ghost-padded, corrected analytically via host-known pad
counts), so the scatter-softmax segment sums become strided windowed
tensor_reduce ops -- no prefix-scan carry chains, no big boundary gathers.
Overflow edges (deg > 32, ~7%) go to a 2048-slot dst-sorted spill region per
graph handled by a small scan + boundary gather.

Per-core layout ("fm" = feature-major packed): edge tensors are
[128 = 16feat x 8graph, 18432 slots]; node tensors [128, 512].
"""

import numpy as np
from contextlib import ExitStack

import concourse.bass as bass
import concourse.bacc as bacc
import concourse.mybir as mybir
import concourse.tile as tile
from concourse import library_config

F32 = mybir.dt.float32
F32R = mybir.dt.float32r
F16 = mybir.dt.float16
F8E4 = mybir.dt.float8e4
I16 = mybir.dt.int16
I8 = mybir.dt.int8

G = 8          # graphs per core
N = 512        # nodes per graph
DEG = 32       # fixed main-slot degree per node
EG = 16384     # edges per graph (input)
EM = N * DEG   # main slots per graph = 16384
ESP = 2048     # spill slots per graph
ET = EM + ESP  # total slots = 18432
EF = 16        # edge/node feature dim after encode
XF = 64        # input node feature dim
K = 256        # topk keep
CHUNK = 1024   # slots per chunk
NCH = ET // CHUNK        # 18 chunks (16 main + 2 spill)
NMAIN = EM // CHUNK      # 16
NT = 528       # gather table columns (512 nodes + ghost pad; 16-multiple)
GHOST = 512
NEND = 576     # padded spill end-list length (513 used)
EPS = 1e-7
BIGNEG = 6.0e4
LN32 = float(np.log(DEG))

import os as _os
STOP_STAGE = _os.environ.get("K_STOP_STAGE") or None  # hw bisect hook


# ----------------------------------------------------------------------------
# Host-side preprocessing: full inputs -> per-core named arrays
# ----------------------------------------------------------------------------

def prep_inputs(inputs: dict) -> list[dict]:
    x = np.asarray(inputs["x"], np.float32)            # [B*N, 64]
    ei = np.asarray(inputs["edge_index"])              # [2, E] int64
    ea = np.asarray(inputs["edge_attr"], np.float32)   # [E, 16]
    B = 64
    assert x.shape == (B * N, XF)
    assert ea.shape == (B * EG, EF)

    src_g = (ei[0] % N).astype(np.int64)
    dst_g = (ei[1] % N).astype(np.int64)
    graph_of_edge = (ei[0] // N).astype(np.int64)
    assert np.array_equal(graph_of_edge, np.repeat(np.arange(B), EG)), \
        "edge blocks not per-graph; prep assumes reference setup_inputs layout"
    assert np.array_equal(ei[0] // N, ei[1] // N)

    def lin(name):
        return np.asarray(inputs[name], np.float32)

    W_ne, b_ne = lin("W_ne"), lin("b_ne")
    W_ee, b_ee = lin("W_ee"), lin("b_ee")
    W1a, b1a, g1, be1 = lin("W1a"), lin("b1a"), lin("g1"), lin("be1")
    W1b, b1b = lin("W1b"), lin("b1b")
    W2a, b2a, g2, be2 = lin("W2a"), lin("b2a"), lin("g2"), lin("be2")
    W2b, b2b = lin("W2b"), lin("b2b")
    Wa, ba, Wo, bo = lin("Wa"), lin("ba"), lin("Wo"), lin("bo")
    w_pool = lin("w_pool")
    wp = w_pool / np.linalg.norm(w_pool)
    t1 = np.float32(inputs["t1"])
    t2 = np.float32(inputs["t2"])

    # centering fold: LN(y) uses yC = y - mean(y) = u @ (W C) + b C
    C32 = np.eye(32, dtype=np.float32) - 1.0 / 32.0
    W1aC = W1a @ C32
    b1aC = b1a @ C32
    W2aC = W2a @ C32
    b2aC = b2a @ C32

    cst = {}
    wne = np.zeros((128, 4 * 128), np.float32)
    for p in range(4):
        for a in range(2):
            gg = 2 * p + a
            wne[64 * a:64 * a + XF, 128 * p + 16 * gg:128 * p + 16 * gg + EF] = W_ne
    cst["wne_stat"] = wne.astype(np.float16)
    cst["bne_vec"] = np.tile(b_ne, G)[:, None].astype(np.float32)
    wee = np.zeros((128, 128), np.float32)
    for g in range(G):
        wee[16 * g:16 * g + EF, 16 * g:16 * g + EF] = W_ee
    cst["wee_stat"] = wee.astype(np.float16)
    cst["bee_vec"] = np.tile(b_ee, G)[:, None].astype(np.float32)
    cst["mln32_vec"] = np.full((128, 1), -LN32, np.float32)
    cst["identT"] = np.eye(128, dtype=np.float32)
    cst["ident"] = np.eye(128, dtype=np.float32)
    cst["t1vec"] = np.full((128, 1), t1, np.float32)
    cst["t2vec"] = np.full((128, 1), t2, np.float32)
    w1a = np.zeros((64, 128), np.float32)
    for gg in range(4):
        w1a[16 * gg:16 * gg + 16, 32 * gg:32 * gg + 32] = W1aC
    cst["w1a_stat"] = np.vstack([w1a, w1a])
    cst["b1a_vec"] = np.tile(b1aC, 4)[:, None].astype(np.float32)
    ones32 = np.zeros((128, 4), np.float32)
    for gg in range(4):
        ones32[32 * gg:32 * gg + 32, gg] = 1.0 / 32.0
    cst["ones32h_stat"] = ones32.astype(np.float16)
    onesb32 = np.zeros((4, 128), np.float32)
    for gg in range(4):
        onesb32[gg, 32 * gg:32 * gg + 32] = 1.0
    cst["onesb32_stat"] = onesb32
    cst["g1_vec"] = np.tile(g1, 4)[:, None].astype(np.float32)
    cst["be1_vec"] = np.tile(be1, 4)[:, None].astype(np.float32)
    w1b = np.zeros((64, 32), np.float32)
    for gg in range(2):
        w1b[32 * gg:32 * gg + 32, 16 * gg:16 * gg + 16] = W1b
    cst["w1bh_stat"] = np.vstack([w1b, w1b]).astype(np.float16)
    cst["b1b_vec"] = np.tile(b1b, G)[:, None].astype(np.float32)
    wpool = np.zeros((128, 8), np.float32)
    for g in range(G):
        wpool[16 * g:16 * g + EF, g] = wp
    cst["wpool_stat"] = wpool
    ones16b = np.zeros((8, 128), np.float32)
    for g in range(G):
        ones16b[g, 16 * g:16 * g + EF] = 1.0
    cst["ones16b_stat"] = ones16b.astype(np.float16)
    onesel = np.zeros((8, 8 * 128), np.float32)
    for g in range(8):
        onesel[g, 128 * g:128 * (g + 1)] = 1.0
    cst["onesel_stat"] = onesel
    w2a = np.zeros((64, 128), np.float32)
    for gg in range(4):
        w2a[16 * gg:16 * gg + 16, 32 * gg:32 * gg + 32] = W2aC
    cst["w2a_stat"] = np.vstack([w2a, w2a])
    cst["b2a_vec"] = np.tile(b2aC, 4)[:, None].astype(np.float32)
    cst["g2_vec"] = np.tile(g2, 4)[:, None].astype(np.float32)
    cst["be2_vec"] = np.tile(be2, 4)[:, None].astype(np.float32)
    w2b = np.zeros((64, 64), np.float32)
    for gg in range(2):
        w2b[32 * gg:32 * gg + 32, 32 * gg:32 * gg + 32] = W2b
    cst["w2bh_stat"] = np.vstack([w2b, w2b]).astype(np.float16)
    cst["b2b_vec"] = np.tile(b2b, 4)[:, None].astype(np.float32)
    mbA = np.zeros((8, 128), np.float32)
    mbB = np.zeros((8, 128), np.float32)
    for g in range(4):
        mbA[g, 32 * g:32 * g + 32] = 1.0
        mbB[g + 4, 32 * g:32 * g + 32] = 1.0
    cst["maskbc_statA"] = mbA.astype(np.float16)
    cst["maskbc_statB"] = mbB.astype(np.float16)
    selk = np.zeros((128, 4 * 32), np.float32)
    for gg in range(4):
        selk[32 * gg:32 * gg + 32, 32 * gg:32 * gg + 32] = np.eye(32) / K
    cst["selk_stat"] = selk
    d8 = np.zeros((8, 64), np.float32)
    for g in range(G):
        d8[g, 9 * g] = 1.0   # spreads transpose(mask) onto per-graph columns
    cst["d8_stat"] = d8
    cst["wa_stat"] = Wa.astype(np.float32)
    cst["ba_vec"] = ba[:, None].astype(np.float32)
    cst["wo_stat"] = Wo.astype(np.float32)
    cst["bo2_vec"] = (-2.0 * bo)[:, None].astype(np.float32)
    cst["lneps_vec"] = np.full((4, 1), 1e-5, np.float32)

    core_maps = []
    for core in range(8):
        m = dict(cst)
        gsl = slice(core * G, (core + 1) * G)
        xt = np.zeros((128, 4 * 512), np.float32)
        xs = x.reshape(B, N, XF)[gsl]
        for p in range(4):
            for a in range(2):
                xt[64 * a:64 * a + XF, 512 * p:512 * (p + 1)] = xs[2 * p + a].T
        m["xT"] = xt.astype(np.float16)

        attrT = np.zeros((128, ET), np.float16)
        srcidx = np.zeros((128, ET // 16), np.int16)
        spendidx = np.zeros((128, NEND // 16), np.int16)
        padcnt = np.zeros((128, N), np.float16)
        amat = np.zeros((128, 32 * 512), np.float16)
        for gl in range(G):
            gid = core * G + gl
            s_l = src_g[gid * EG:(gid + 1) * EG]
            d_l = dst_g[gid * EG:(gid + 1) * EG]
            order = np.argsort(d_l, kind="stable")
            ds = d_l[order]
            ss = s_l[order]
            ats = ea[gid * EG:(gid + 1) * EG][order]     # [EG, 16] dst-sorted
            counts = np.bincount(ds, minlength=N)
            starts = np.zeros(N + 1, np.int64)
            np.cumsum(counts, out=starts[1:])
            # main: node n gets its first min(deg,32) edges at slots 32n+o
            j = np.arange(EM)
            nn = j // DEG
            oo = j % DEG
            msk = oo < np.minimum(counts[nn], DEG)
            pos = starts[nn] + oo
            srcf = np.full(ET, GHOST, np.int64)
            attrf = np.zeros((ET, EF), np.float32)
            srcf[j[msk]] = ss[pos[msk]]
            attrf[j[msk]] = ats[pos[msk]]
            # spill: rank within dst-run >= 32 (already dst-sorted)
            r = np.arange(EG) - starts[ds]
            spm = r >= DEG
            nsp = int(spm.sum())
            assert nsp <= ESP, f"spill overflow: {nsp} > {ESP}"
            srcf[EM:EM + nsp] = ss[spm]
            attrf[EM:EM + nsp] = ats[spm]
            spd = ds[spm]
            e_sp = np.searchsorted(spd, np.arange(N), side="right")
            elist = np.zeros(NEND, np.int16)
            elist[1:N + 1] = e_sp.astype(np.int16)

            attrT[16 * gl:16 * gl + EF, :] = attrf.T.astype(np.float16)
            srcidx[16 * gl:16 * gl + 16, :] = \
                srcf.astype(np.int16).reshape(ET // 16, 16).T
            spendidx[16 * gl:16 * gl + 16, :] = \
                elist.reshape(NEND // 16, 16).T
            padcnt[16 * gl:16 * gl + 16, :] = \
                np.maximum(0, DEG - counts)[None, :].astype(np.float16)
            # adjacency count matrix A[m, n] = #edges m->n (for the L2
            # masked-src denominator correction via matmul)
            A = np.zeros((N, N), np.int32)
            np.add.at(A, (s_l, d_l), 1)
            assert A.max() <= 2048, A.max()  # f16 integers exact to 2048
            for b in range(4):
                amat[:, (4 * gl + b) * 512:(4 * gl + b + 1) * 512] = \
                    A[128 * b:128 * (b + 1), :].astype(np.float16)
        m["spendidx"] = spendidx
        m["padcnt"] = padcnt
        blob = np.zeros((128, CBLOB_BYTES), np.uint8)
        for name, shape, dt, off in CONST_SPECS:
            arr = m[name]
            bv = arr.view(np.uint8).reshape(arr.shape[0], -1)
            blob[:arr.shape[0], off:off + bv.shape[1]] = bv
        rblob = np.zeros((128, 4 * 128), np.float32)
        rblob[:, 0:128] = m["ident"]
        rblob[:, 128:256] = m["w1a_stat"]
        rblob[:, 256:384] = m["w2a_stat"]
        rblob[0:4, 384:512] = m["onesb32_stat"]
        core_maps.append({"cblob": blob, "rblob": rblob, "attrT": attrT,
                          "xT": m["xT"], "srcidx": srcidx, "amat": amat})
    return core_maps


_CSPEC_RAW = [
    # encode-critical constants first (covered by the first cblob DMA piece)
    ("wne_stat", [128, 4 * 128], F16),
    ("bne_vec", [128, 1], F32),
    ("wee_stat", [128, 128], F16),
    ("bee_vec", [128, 1], F32),
    ("mln32_vec", [128, 1], F32),
    ("t1vec", [128, 1], F32),
    ("t2vec", [128, 1], F32),
    ("spendidx", [128, NEND // 16], I16),
    ("padcnt", [128, N], F16),
    # ---- split point: everything below arrives with the second DMA ----
    ("identT", [128, 128], F32),
    ("b1a_vec", [128, 1], F32),
    ("ones32h_stat", [128, 4], F16),
    ("g1_vec", [128, 1], F32),
    ("be1_vec", [128, 1], F32),
    ("w1bh_stat", [128, 32], F16),
    ("b1b_vec", [128, 1], F32),
    ("wpool_stat", [128, 8], F32),
    ("ones16b_stat", [8, 128], F16),
    ("onesel_stat", [8, 8 * 128], F32),
    ("b2a_vec", [128, 1], F32),
    ("g2_vec", [128, 1], F32),
    ("be2_vec", [128, 1], F32),
    ("w2bh_stat", [128, 64], F16),
    ("b2b_vec", [128, 1], F32),
    ("maskbc_statA", [8, 128], F16),
    ("maskbc_statB", [8, 128], F16),
    ("selk_stat", [128, 4 * 32], F32),
    ("d8_stat", [8, 64], F32),
    ("wa_stat", [32, 128], F32),
    ("ba_vec", [128, 1], F32),
    ("wo_stat", [128, 4], F32),
    ("bo2_vec", [4, 1], F32),
    ("lneps_vec", [4, 1], F32),
]

_SPLIT_AFTER = "padcnt"


def _mk_const_specs():
    specs = []
    off = 0
    for nm, shape, dt in _CSPEC_RAW:
        nbytes = int(np.prod(shape[1:])) * mybir.dt.size(dt)
        specs.append((nm, shape, dt, off))
        off += (nbytes + 127) // 128 * 128
    return specs, off

CONST_SPECS, CBLOB_BYTES = _mk_const_specs()

INPUT_SPECS = [
    ("cblob", [128, CBLOB_BYTES], mybir.dt.uint8),
    ("rblob", [128, 4 * 128], F32R),   # ident | w1a | w2a | onesb32(rows 0:4)
    ("attrT", [128, ET], F16),
    ("xT", [128, 4 * 512], F16),
    ("srcidx", [128, ET // 16], I16),
    ("amat", [128, 32 * 512], F16),
]


# ----------------------------------------------------------------------------
# Device graph
# ----------------------------------------------------------------------------

def build_nc(debug_keys=(), n_rep=1):
    nc = bacc.Bacc(None, target_bir_lowering=False, debug=False)
    A = {}
    for name, shape, dt in INPUT_SPECS:
        A[name] = nc.declare_dram_parameter(name, shape, dt, isOutput=False)[:]
    out_ext = nc.declare_dram_parameter("out", [4, G], F32, isOutput=True)[:]
    dbg_ext = {}
    dbg_shapes = {
        "hT": ((128, N), F32), "mc0": ((128, CHUNK), F16),
        "ev0": ((128, CHUNK), F16), "evm0": ((128, CHUNK), F16),
        "sums0": ((128, N), F16), "sums1": ((128, N), F16),
        "spev0": ((128, ESP), F16), "Ssp0": ((128, ESP + 1), F32),
        "Dsp0": ((128, N), F32), "Dsp1": ((128, N), F32),
        "aggr1": ((128, N), F32), "u1": ((128, N), F32),
        "h1": ((128, N), F32),
        "score": ((8, N), F32), "rk": ((128, 32), F32),
        "mask": ((128, 32), F32),
        "hq": ((128, N), F32), "hp": ((128, N), F32),
        "aggr2": ((128, N), F32), "h2a": ((128, N), F16),
        "h2b": ((128, N), F16), "pooled": ((32, 8), F32),
        "y1sA": ((128, N), F16), "r1A": ((128, N), F16),
    }
    for key in debug_keys:
        shape, dt = dbg_shapes[key]
        dbg_ext[key] = nc.declare_dram_parameter(
            "dbg_" + key, list(shape), dt, isOutput=True)[:]

    with tile.TileContext(nc) as tc, ExitStack() as ctx:
        consts = ctx.enter_context(tc.tile_pool(name="consts", bufs=1))
        nodep = ctx.enter_context(tc.tile_pool(name="nodep", bufs=1))
        work = ctx.enter_context(tc.tile_pool(name="work", bufs=2))
        psum = ctx.enter_context(tc.tile_pool(name="psum", bufs=2, space="PSUM"))
        psum1 = ctx.enter_context(tc.tile_pool(name="psum1", bufs=1, space="PSUM"))
        psume = ctx.enter_context(tc.tile_pool(name="psume", bufs=1, space="PSUM"))
        psumw = ctx.enter_context(tc.tile_pool(name="psumw", bufs=1, space="PSUM"))

        nc.gpsimd.load_library(library_config.ap_gather)

        # ---- DMAs: xT first (encode-critical), then cblob, srcidx, attr ----
        xts = consts.tile([128, 4 * 512], F16, name="xT_sb")
        nc.sync.dma_start(out=xts, in_=A["xT"])
        cb = consts.tile([128, CBLOB_BYTES], mybir.dt.uint8, name="cblob_sb")
        _split = next(off for nm, _s, _d, off in CONST_SPECS
                      if nm == "identT")
        nc.sync.dma_start(out=cb[:, :_split], in_=A["cblob"][:, :_split])
        srcidx_sb = consts.tile([128, ET // 16], I16, name="srcidx_sb")
        nc.sync.dma_start(out=srcidx_sb, in_=A["srcidx"])
        attr_all = consts.tile([128, ET], F16, name="attr_sb")
        # pieces in chunk-consumption order: main[0:2048], spill, rest
        nc.sync.dma_start(out=attr_all[:, 0:2048], in_=A["attrT"][:, 0:2048])
        rb_sb = consts.tile([128, 4 * 128], F32R, name="rblob_sb")
        nc.sync.dma_start(out=rb_sb, in_=A["rblob"])
        nc.sync.dma_start(out=attr_all[:, EM:], in_=A["attrT"][:, EM:])
        for lo, hi in ((2048, 8192), (8192, 12288), (12288, EM)):
            nc.sync.dma_start(out=attr_all[:, lo:hi], in_=A["attrT"][:, lo:hi])
        nc.sync.dma_start(out=cb[:, _split:], in_=A["cblob"][:, _split:])
        amat_sb = consts.tile([128, 16 * 512], F16, name="amat_sb")
        nc.sync.dma_start(out=amat_sb, in_=A["amat"][:, :16 * 512])

        C = {"srcidx": srcidx_sb, "attr": attr_all, "amat": amat_sb,
             "amat_dram": A["amat"],
             "ident": rb_sb[:, 0:128], "w1a_stat": rb_sb[:, 128:256],
             "w2a_stat": rb_sb[:, 256:384],
             "onesb32_stat": rb_sb[0:4, 384:512]}
        for name, shape, dt, off in CONST_SPECS:
            nbytes = int(np.prod(shape[1:])) * mybir.dt.size(dt)
            ap = cb[:shape[0], off:off + nbytes].bitcast(dt)
            if len(shape) == 3:
                ap = ap.rearrange("p (a b) -> p a b", b=shape[2])
            C[name] = ap

        dbg_keys_set = set(debug_keys)

        def dbg(key, ap):
            if key in dbg_keys_set:
                nc.sync.dma_start(out=dbg_ext[key], in_=ap)

        for rep in range(n_rep):
            run_once(nc, tc, A, C, xts, out_ext, dbg, nodep, work,
                     psum, psum1, psume, psumw, rep)
    nc.compile()
    return nc


def run_once(nc, tc, A, C, xts, out_ext, dbg, nodep, work, psum, psum1,
             psume, psumw, rep):
    V = nc.vector
    S = nc.scalar
    T = nc.tensor
    Alu = mybir.AluOpType
    Act = mybir.ActivationFunctionType

    # ================= node encode: hT = x @ W_ne + b_ne =================
    hraw = psume.tile([128, N], F32, name="hraw", tag="e512")
    for p in range(4):
        T.matmul(hraw, C["wne_stat"][:, 128 * p:128 * (p + 1)],
                 xts[:, 512 * p:512 * (p + 1)],
                 start=(p == 0), stop=(p == 3))
    hT = nodep.tile([128, NT, 1], F32, name=f"hT_{rep}")
    S.activation(hT[:, :N, 0], hraw, Act.Identity, bias=C["bne_vec"], scale=1.0)
    S.activation(hT[:, N:, 0], hraw[:, 0:NT - N], Act.Copy, bias=0.0, scale=0.0)
    dbg("hT", hT[:, :N, 0])

    def stop_dma(ap):
        nc.sync.dma_start(out=out_ext, in_=ap)

    if STOP_STAGE == "enc":
        return stop_dma(hT[0:4, 0:G, 0].bitcast(F32))

    # ================= shared edge-phase machinery =================
    def edge_layer(layer, table, tvec, bee, mm=None, dbg_on=False,
                   inject=None):
        """Returns aggr tile [128, N] f32 (softmax-weighted mean + pad fix)."""
        sums = nodep.tile([128, 2, N], F32, name=f"sums_{layer}_{rep}",
                          tag="sums")
        spev = nodep.tile([128, 2, ESP], F16, name=f"spev_{layer}_{rep}",
                          tag="spev")
        # ev for ghost-pad slots: mc_pad = relu(bee); ev_pad = exp(t*mc - ln32)
        beer = nodep.tile([128, 1], F32, name=f"beer_{layer}_{rep}", tag="beer")
        S.activation(beer, bee, Act.Relu, bias=0.0, scale=1.0)
        evpad = nodep.tile([128, 1], F32, name=f"evpad_{layer}_{rep}",
                           tag="evpad")
        S.activation(evpad, beer, Act.Exp, bias=C["mln32_vec"], scale=tvec)
        evmpad = nodep.tile([128, 1], F32, name=f"evmpad_{layer}_{rep}",
                            tag="evmpad")
        V.tensor_tensor(out=evmpad, in0=evpad, in1=beer, op=Alu.mult)

        # 2 main chunks prime the pipeline, then spill (so its scan +
        # boundary-gather tail overlaps the remaining main chunks)
        E = (sums, None, evpad, evmpad)
        for cc in [0, 1, NMAIN, NMAIN + 1] + list(range(2, NMAIN)):
            if inject is not None and cc == inject[0]:
                E = (sums, Dsp, evpad, evmpad)
                inject[1](E)
            base = cc * CHUNK
            hsrc = work.tile([128, CHUNK, 1], F32, name="hsrc", tag="hsrc")
            j0 = base // 16
            nc.gpsimd.ap_gather(
                hsrc, table, C["srcidx"][:, j0:j0 + CHUNK // 16],
                channels=128, num_elems=NT, d=1, num_idxs=CHUNK)
            if STOP_STAGE == "pipe1a" and layer == 0:
                return None
            zc = psum.tile([128, CHUNK], F32, name="zc", tag="zc")
            for s in range(2):
                sl = slice(512 * s, 512 * (s + 1))
                T.matmul(zc[:, sl], C["wee_stat"],
                         C["attr"][:, base + 512 * s:base + 512 * (s + 1)],
                         start=True, stop=False, skip_group_check=True)
                T.matmul(zc[:, sl], C["ident"].bitcast(F32), hsrc[:, sl, 0],
                         start=False, stop=True, skip_group_check=True)
            mc = work.tile([128, CHUNK], F16, name="mc", tag="mc")
            S.activation(mc, zc, Act.Relu, bias=bee, scale=1.0)
            if STOP_STAGE == "pipe1b" and layer == 0:
                return None
            if cc < NMAIN:
                evcc = work.tile([128, 2, CHUNK], F16, name="evcc", tag="evcc")
                ev_t, evm_t = evcc[:, 0, :], evcc[:, 1, :]
            else:
                sp = cc - NMAIN
                spsl = slice(CHUNK * sp, CHUNK * (sp + 1))
                ev_t, evm_t = spev[:, 0, spsl], spev[:, 1, spsl]
            # L2 masked-src edges (hq[src] = -2e9): mc = 0 exactly, so they
            # add 0 to the numerator and exactly 1/32 to the denominator;
            # the denominator excess is removed analytically via `mm`
            # (adjacency matmul) instead of a per-edge mask multiply.
            S.activation(ev_t, mc, Act.Exp, bias=C["mln32_vec"], scale=tvec)
            V.tensor_tensor(out=evm_t, in0=ev_t, in1=mc, op=Alu.mult)
            if STOP_STAGE == "pipe1" and layer == 0:
                return None
            if cc < NMAIN:
                # windowed segment-sum via pairwise folds (f16 2x DVE mode)
                # + small f32 reduce: ~1550ns vs 2254 for one big 1x reduce
                f1 = work.tile([128, 2, 32, 16], F16, name="f1", tag="f1")
                ev4 = evcc[:, :, :].rearrange("p a (b c) -> p a b c", c=DEG)
                V.tensor_tensor(out=f1, in0=ev4[:, :, :, 0:16],
                                in1=ev4[:, :, :, 16:32], op=Alu.add)
                f2 = work.tile([128, 2, 32, 8], F16, name="f2", tag="f2")
                V.tensor_tensor(out=f2, in0=f1[:, :, :, 0:8],
                                in1=f1[:, :, :, 8:16], op=Alu.add)
                f3 = work.tile([128, 2, 32, 4], F16, name="f3", tag="f3")
                V.tensor_tensor(out=f3, in0=f2[:, :, :, 0:4],
                                in1=f2[:, :, :, 4:8], op=Alu.add)
                V.tensor_reduce(
                    out=sums[:, :, DEG * cc:DEG * (cc + 1)],
                    in_=f3, axis=mybir.AxisListType.X, op=Alu.add)
            if dbg_on and cc == 0:
                dbg("mc0", mc)
                dbg("ev0", ev_t)
                dbg("evm0", evm_t)
            if cc == NMAIN + 1:
                # spill scans + boundary gather (overlap with main chunks)
                Dsp = []
                for ti in range(2):
                    Ssp = nodep.tile([128, ESP + 1, 1], F32,
                                     name=f"Ssp{ti}_{layer}_{rep}",
                                     tag=f"Ssp{ti}")
                    V.memset(Ssp[:, 0:1, 0], 0.0)
                    V.tensor_tensor_scan(
                        out=Ssp[:, 1:, 0], data0=spev[:, ti, :],
                        data1=spev[:, ti, :], initial=0.0,
                        op0=Alu.add, op1=Alu.bypass)
                    gsp = nodep.tile([128, NEND, 1], F32,
                                     name=f"gsp{ti}_{layer}_{rep}",
                                     tag=f"gsp{ti}")
                    nc.gpsimd.ap_gather(
                        gsp, Ssp, C["spendidx"],
                        channels=128, num_elems=ESP + 1, d=1, num_idxs=NEND)
                    d = nodep.tile([128, N], F32, name=f"Dsp{ti}_{layer}_{rep}",
                                   tag=f"Dsp{ti}")
                    V.tensor_tensor(out=d, in0=gsp[:, 1:N + 1, 0],
                                    in1=gsp[:, 0:N, 0], op=Alu.subtract)
                    Dsp.append(d)
                if dbg_on:
                    dbg("spev0", spev[:, 0, :])
                    dbg("Ssp0", Ssp[:, :, 0])
                    dbg("Dsp0", Dsp[0])
                    dbg("Dsp1", Dsp[1])

        if dbg_on:
            dbg("sums0", sums[:, 0, :])
            dbg("sums1", sums[:, 1, :])
        return (sums, Dsp, evpad, evmpad)

    def aggr_half(E, hs, base_in, mm, layer, hi):
        """Softmax-mean for node slice hs; returns u = base + aggr + EPS."""
        sums, Dsp, evpad, evmpad = E
        HN = hs.stop - hs.start
        neg = []
        for ti, padv in ((0, evpad), (1, evmpad)):
            tot = nodep.tile([128, HN], F32, name=f"tot{ti}_{layer}{hi}_{rep}",
                             tag=f"tot{ti}")
            V.tensor_tensor(out=tot, in0=sums[:, ti, hs], in1=Dsp[ti][:, hs],
                            op=Alu.add)
            if ti == 0 and mm is not None:
                # tot += (mask-1)^T A / 32 (mm pre-scaled by 1/32)
                mmb = psume.tile([128, HN], F32, name="mmb", tag="e512")
                T.matmul(mmb, C["ones16b_stat"], mm[:, hs],
                         start=True, stop=True)
                tot2 = nodep.tile([128, HN], F32,
                                  name=f"tot2_{layer}{hi}_{rep}", tag="tot2")
                V.tensor_tensor(out=tot2, in0=mmb, in1=tot, op=Alu.add)
                tot = tot2
            ng = nodep.tile([128, HN], F32, name=f"ng{ti}_{layer}{hi}_{rep}",
                            tag=f"ng{ti}")
            V.scalar_tensor_tensor(out=ng, in0=C["padcnt"][:, hs], scalar=padv,
                                   in1=tot, op0=Alu.mult, op1=Alu.subtract)
            neg.append(ng)
        dm = nodep.tile([128, HN], F32, name=f"dm_{layer}{hi}_{rep}", tag="dm")
        V.tensor_scalar(dm, neg[0], -1e-16, None, Alu.min)
        rec = nodep.tile([128, HN], F32, name=f"rec_{layer}{hi}_{rep}",
                         tag="rec")
        V.reciprocal(rec, dm)
        ag = nodep.tile([128, HN], F32, name=f"ag_{layer}{hi}_{rep}", tag="ag")
        V.tensor_tensor(out=ag, in0=neg[1], in1=rec, op=Alu.mult)
        u = nodep.tile([128, HN], F32R, name=f"u_{layer}{hi}_{rep}", tag="u")
        V.scalar_tensor_tensor(out=u, in0=ag, scalar=EPS, in1=base_in[:, hs],
                               op0=Alu.add, op1=Alu.add)
        return u

    def mlp_half(uin, hs, wa_stat, ba_vec, gvec, bevec, wb_stat, layer, hi):
        HN = hs.stop - hs.start
        N2 = 2 * HN
        y1p = psumw.tile([128, 2, HN], F32, name=f"y1p_{layer}{hi}",
                         tag="wide")
        for half in range(2):
            T.matmul(y1p[:, half, :],
                     wa_stat[64 * half:64 * half + 64, :],
                     uin[64 * half:64 * half + 64, :],
                     start=True, stop=True)
        y1pf = y1p.rearrange("p a b -> p (a b)")
        y1s = nodep.tile([128, N2], F16, name=f"y1s_{layer}{hi}_{rep}",
                         tag="y1s")
        S.activation(y1s, y1pf, Act.Identity, bias=ba_vec, scale=1.0)
        sq = nodep.tile([128, N2], F16, name=f"sq_{layer}{hi}_{rep}", tag="sq")
        V.tensor_tensor(out=sq, in0=y1s, in1=y1s, op=Alu.mult)
        vp = psumw.tile([4, 2, HN], F32, name=f"vp_{layer}{hi}", tag="wide")
        for half in range(2):
            T.matmul(vp[:, half, :], C["ones32h_stat"],
                     sq[:, half * HN:half * HN + HN], start=True, stop=True)
        lnv = nodep.tile([4, N2], F32, name=f"lnv_{layer}{hi}_{rep}",
                         tag="st4", bufs=2)
        S.activation(lnv, vp.rearrange("p a b -> p (a b)"), Act.Ln,
                     bias=C["lneps_vec"], scale=1.0)
        rstd = nodep.tile([4, N2], F32R, name=f"rstd_{layer}{hi}_{rep}",
                          tag="st4", bufs=2)
        S.activation(rstd, lnv, Act.Exp, bias=0.0, scale=-0.5)
        rb = psumw.tile([128, 2, HN], F32, name=f"rb_{layer}{hi}", tag="wide")
        for half in range(2):
            T.matmul(rb[:, half, :], C["onesb32_stat"],
                     rstd[:, half * HN:half * HN + HN],
                     start=True, stop=True)
        vnorm = nodep.tile([128, N2], F32, name=f"vn_{layer}{hi}_{rep}",
                           tag="vn")
        V.tensor_tensor(out=vnorm, in0=y1s,
                        in1=rb.rearrange("p a b -> p (a b)"), op=Alu.mult)
        r1 = nodep.tile([128, N2], F16, name=f"r1_{layer}{hi}_{rep}", tag="r1")
        S.activation(r1, vnorm, Act.Relu, bias=bevec, scale=gvec)
        M = wb_stat.shape[1]
        outs = []
        for half in range(2):
            for q in range(2):
                yq = psum.tile([M, HN], F32, name=f"yq{half}{q}", tag="zc")
                T.matmul(yq, wb_stat[64 * q:64 * q + 64, :],
                         r1[64 * q:64 * q + 64, half * HN:half * HN + HN],
                         start=True, stop=True)
                outs.append(yq)
        return outs

    HALVES = (slice(0, N),)

    # ================= Layer 1 =================
    h1 = nodep.tile([128, N], F32, name=f"h1_{rep}", tag="h1")

    def half_pipe(layer, base_in, mm, wa, ba, gv, bev, wb, hout):
        def emit(E, hi, hs):
            u = aggr_half(E, hs, base_in, mm, layer, hi)
            y2q = mlp_half(u, hs, wa, ba, gv, bev, wb, layer, hi)
            for q in range(4):
                if layer == 0:
                    S.activation(hout[32 * q:32 * q + 32, hs], y2q[q],
                                 Act.Relu,
                                 bias=C["b1b_vec"][32 * q:32 * q + 32, :],
                                 scale=1.0)
                else:
                    sl, part = q // 2, q % 2
                    S.activation(hout[sl][64 * part:64 * part + 64, hs],
                                 y2q[q], Act.Relu,
                                 bias=C["b2b_vec"][64 * part:64 * part + 64, :],
                                 scale=1.0)
        return emit

    emit1 = half_pipe(0, hT[:, :N, 0], None, C["w1a_stat"], C["b1a_vec"],
                      C["g1_vec"], C["be1_vec"], C["w1bh_stat"], h1)
    E1 = edge_layer(0, hT, C["t1vec"], C["bee_vec"], dbg_on=True)
    if STOP_STAGE in ("pipe1", "pipe1a", "pipe1b"):
        return None
    emit1(E1, 0, HALVES[0])
    dbg("h1", h1)
    if STOP_STAGE == "mlp1":
        return stop_dma(h1[0:4, 0:G])

    # ================= score / topk mask / gates =================
    scp = psume.tile([8, N], F32, name="scp", tag="e512")
    T.matmul(scp, C["wpool_stat"], h1, start=True, stop=True)
    scs = nodep.tile([8, N], F32, name=f"scs_{rep}", tag="scs")
    S.activation(scs, scp, Act.Copy, bias=0.0, scale=1.0)
    dbg("score", scs)
    snode = nodep.tile([128, 4, 8], F32, name=f"snode_{rep}", tag="snode")
    for t in range(4):
        tp = psum1.tile([128, 8], F32, name="tp", tag="small")
        T.transpose(tp, scs[:, 128 * t:128 * (t + 1)], C["identT"][0:8, 0:8])
        S.activation(snode[:, t, :], tp, Act.Copy, bias=0.0, scale=1.0)
    sneg = nodep.tile([128, 4, 8], F32, name=f"sneg_{rep}", tag="sneg")
    V.tensor_scalar(sneg, snode, -1.0, None, Alu.mult)
    # rank: graphs 0-3 on Act (sign-sum), graphs 4-7 on DVE (is_gt count)
    rk = nodep.tile([128, 4, 8], F32, name=f"rk_{rep}", tag="rk")
    # interleave Act-half (g<4) and DVE-half (g>=4) so both engines run
    # concurrently despite the 2-deep sb psum ring
    for g in (0, 4, 1, 5, 2, 6, 3, 7):
        sb = psum.tile([128, N], F32, name="sb", tag="zc")
        T.matmul(sb, C["onesel_stat"][:, 128 * g:128 * (g + 1)],
                 scs, start=True, stop=True)
        if g < 4:
            for t in range(4):
                sga = work.tile([128, N], F16, name="sga", tag="sga")
                S.activation(sga, sb, Act.Sign, bias=sneg[:, t, g:g + 1],
                             scale=1.0, accum_out=rk[:, t, g:g + 1])
        else:
            for t in range(4):
                sgv = work.tile([128, N], F16, name="sgv", tag="sgv")
                V.tensor_scalar(sgv, sb, snode[:, t, g:g + 1], 0.0,
                                Alu.is_gt, Alu.add,
                                accum_out=rk[:, t, g:g + 1])
    dbg("rk", rk.rearrange("p a b -> p (a b)"))
    if STOP_STAGE == "rank":
        return stop_dma(rk[0:4, 0, 0:G])
    mask01 = nodep.tile([128, 4, 8], F32, name=f"mask01_{rep}", tag="mask01")
    V.tensor_scalar(mask01[:, :, 0:4], rk[:, :, 0:4], -1.0, None, Alu.is_le)
    V.tensor_scalar(mask01[:, :, 4:8], rk[:, :, 4:8], float(K) - 0.5, None,
                    Alu.is_le)
    dbg("mask", mask01.rearrange("p a b -> p (a b)"))
    ex = nodep.tile([128, 4, 8], F32, name=f"ex_{rep}", tag="ex")
    S.activation(ex, snode, Act.Exp, bias=0.0, scale=-2.0)
    exd = nodep.tile([128, 4, 8], F32, name=f"exd_{rep}", tag="exd")
    V.tensor_scalar(exd, ex, 1.0, None, Alu.add)
    exr = nodep.tile([128, 4, 8], F32, name=f"exr_{rep}", tag="exr")
    V.reciprocal(exr, exd)
    th = nodep.tile([128, 4, 8], F32, name=f"th_{rep}", tag="th")
    V.tensor_scalar(th, exr, 2.0, -1.0, Alu.mult, Alu.add)
    gate = nodep.tile([128, 4, 8], F32, name=f"gate_{rep}", tag="gate")
    V.tensor_tensor(out=gate, in0=th, in1=mask01, op=Alu.mult)
    gq = nodep.tile([128, 4, 8], F32, name=f"gq_{rep}", tag="gq")
    V.tensor_scalar(gq, mask01, -1.0, BIGNEG, Alu.add, Alu.mult)
    gfm = nodep.tile([8, N], F16, name=f"gfm_{rep}", tag="gfm")
    qfm = nodep.tile([8, N], F16, name=f"qfm_{rep}", tag="qfm")
    mfm = nodep.tile([8, N], F16, name=f"mfm_{rep}", tag="mfm")
    for t in range(4):
        tstack = work.tile([128, 96], F32, name="tstack", tag="tstack")
        V.tensor_copy(out=tstack[:, 0:8], in_=gate[:, t, :])
        V.tensor_copy(out=tstack[:, 32:40], in_=gq[:, t, :])
        V.tensor_copy(out=tstack[:, 64:72], in_=mask01[:, t, :])
        tq = psum1.tile([96, 128], F32, name="tq", tag="small")
        T.transpose(tq, tstack, C["identT"])
        S.activation(gfm[:, 128 * t:128 * (t + 1)], tq[0:8, :], Act.Copy,
                     bias=0.0, scale=1.0)
        S.activation(qfm[:, 128 * t:128 * (t + 1)], tq[32:40, :], Act.Copy,
                     bias=0.0, scale=1.0)
        S.activation(mfm[:, 128 * t:128 * (t + 1)], tq[64:72, :], Act.Copy,
                     bias=0.0, scale=1.0)
    # masked-src edge counts: mmg[g, n] = sum_m (mask_g[m]-1) * A_g[m, n];
    # per (g, block) lhsT is [128, 8] with only column g nonzero, so all 32
    # matmuls accumulate into one [8, N] psum (zero rows elsewhere)
    mfm1 = nodep.tile([8, N], F32, name=f"mfm1_{rep}", tag="mfm1")
    V.tensor_scalar(mfm1, mfm, -1.0, None, Alu.add)
    zsb = nodep.tile([128, 4, 64], F16, name=f"zsb_{rep}", tag="zsb")
    for b in range(4):
        zp = psum1.tile([128, 64], F32, name="zp", tag="small")
        T.matmul(zp, mfm1[:, 128 * b:128 * (b + 1)], C["d8_stat"],
                 start=True, stop=True)
        S.activation(zsb[:, b, :], zp, Act.Copy, bias=0.0, scale=1.0)
    mmg_ps = psumw.tile([8, N], F32, name="mmg_ps", tag="wide")
    for g in range(G):
        if g == 4:   # second half of A overwrites the buffer (WAR-tracked)
            nc.sync.dma_start(out=C["amat"], in_=C["amat_dram"][:, 16 * 512:])
        for b in range(4):
            T.matmul(mmg_ps, zsb[:, b, 8 * g:8 * g + 8],
                     C["amat"][:, ((g % 4) * 4 + b) * 512:
                               ((g % 4) * 4 + b + 1) * 512],
                     start=(g == 0 and b == 0), stop=(g == 7 and b == 3))
    mmg_s = nodep.tile([8, N], F16, name=f"mmg_s_{rep}", tag="mmg_s")
    S.activation(mmg_s, mmg_ps, Act.Copy, bias=0.0, scale=1.0 / DEG)

    gb = psume.tile([128, N], F32, name="gb", tag="e512")
    T.matmul(gb, C["ones16b_stat"], gfm, start=True, stop=True)
    hp = nodep.tile([128, N], F32, name=f"hp_{rep}", tag="hp")
    V.tensor_tensor(out=hp, in0=h1, in1=gb, op=Alu.mult)
    dbg("hp", hp)
    qb = psume.tile([128, N], F32, name="qb", tag="e512")
    T.matmul(qb, C["ones16b_stat"], qfm, start=True, stop=True)
    hq = nodep.tile([128, NT, 1], F32, name=f"hq_{rep}", tag="hq")
    V.tensor_tensor(out=hq[:, :N, 0], in0=hp, in1=qb, op=Alu.add)
    S.activation(hq[:, N:, 0], qb[:, 0:NT - N], Act.Copy, bias=0.0, scale=0.0)
    dbg("hq", hq[:, :N, 0])
    if STOP_STAGE == "hq":
        return stop_dma(hq[0:4, 0:G, 0].bitcast(F32))

    # ================= Layer 2 =================
    h2 = [nodep.tile([128, N], F16, name=f"h2{sl}_{rep}", tag=f"h2{sl}")
          for sl in range(2)]
    emit2 = half_pipe(1, hp, mmg_s, C["w2a_stat"], C["b2a_vec"],
                      C["g2_vec"], C["be2_vec"], C["w2bh_stat"], h2)
    E2 = edge_layer(1, hq, C["t2vec"], C["bee_vec"], mm=mmg_s)
    if STOP_STAGE == "l2agg":
        return stop_dma(mmg_s[0:4, 0:G])
    emit2(E2, 0, HALVES[0])
    dbg("h2a", h2[0])
    dbg("h2b", h2[1])

    # ================= pooling + head =================
    pooled = []
    for sl, statname in ((0, "maskbc_statA"), (1, "maskbc_statB")):
        mb2 = psume.tile([128, N], F32, name=f"mbp{sl}", tag="e512")
        T.matmul(mb2, C[statname], mfm, start=True, stop=True)
        mbh = nodep.tile([128, N], F16, name=f"mbh{sl}_{rep}", tag=f"mbh{sl}")
        S.activation(mbh, mb2, Act.Copy, bias=0.0, scale=1.0)
        pl = nodep.tile([128, 1], F32, name=f"pl{sl}_{rep}", tag=f"pl{sl}")
        scratch = work.tile([128, N], F16, name="plscratch", tag="plscratch",
                            bufs=1)
        V.scalar_tensor_tensor(out=scratch, in0=h2[sl], scalar=1.0, in1=mbh,
                               op0=Alu.mult, op1=Alu.mult, accum_out=pl)
        pooled.append(pl)
    P8 = psum1.tile([32, G], F32, name="P8", tag="small")
    for g in range(G):
        sl, gg = g // 4, g % 4
        T.matmul(P8[:, g:g + 1],
                 C["selk_stat"][:, 32 * gg:32 * gg + 32],
                 pooled[sl], start=True, stop=True,
                 skip_group_check=True)
    p8s = nodep.tile([32, G], F32, name=f"p8s_{rep}", tag="p8s")
    S.activation(p8s, P8, Act.Copy, bias=0.0, scale=1.0)
    dbg("pooled", p8s)
    a1p = psume.tile([128, G], F32, name="a1p", tag="e512")
    T.matmul(a1p, C["wa_stat"], p8s, start=True, stop=True)
    a1 = nodep.tile([128, G], F32, name=f"a1_{rep}", tag="a1")
    S.activation(a1, a1p, Act.Relu, bias=C["ba_vec"], scale=1.0)
    op = psum1.tile([4, G], F32, name="op", tag="small")
    T.matmul(op, C["wo_stat"], a1, start=True, stop=True)
    oe = nodep.tile([4, G], F32, name=f"oe_{rep}", tag="oe")
    S.activation(oe, op, Act.Exp, bias=C["bo2_vec"], scale=-2.0)
    od = nodep.tile([4, G], F32, name=f"od_{rep}", tag="od")
    V.tensor_scalar(od, oe, 1.0, None, Alu.add)
    orr = nodep.tile([4, G], F32, name=f"orr_{rep}", tag="orr")
    V.reciprocal(orr, od)
    ot = nodep.tile([4, G], F32, name=f"ot_{rep}", tag="ot")
    V.tensor_scalar(ot, orr, 2.0, -1.0, Alu.mult, Alu.add)
    nc.sync.dma_start(out=out_ext, in_=ot)


# ----------------------------------------------------------------------------
# Self-contained entry point: kernel(**inputs) -> [64, 4] float32
# ----------------------------------------------------------------------------
import jax as _jax
from jax.sharding import Mesh as _Mesh, PartitionSpec as _PartitionSpec
from jax.experimental.shard_map import shard_map as _shard_map

_COMPILED = {}


def _build_and_jit():
    """Re-create the jitted executable on every call: re-executing a loaded
    NEFF leaves device state (semaphores) behind and corrupts the second run,
    so each kernel() invocation gets a fresh executable (BIR->NEFF is
    disk-cached, so this costs seconds, not a recompile)."""
    from concourse import bass2jax
    from concourse.bass2jax import _bass_exec_p, partition_id_tensor

    if "nc" in _COMPILED:
        nc = _COMPILED["nc"]
    else:
        nc = build_nc()
        _COMPILED["nc"] = nc
    bass2jax.install_neuronx_cc_hook()
    partition_name = (nc.partition_id_tensor.name
                      if nc.partition_id_tensor else None)
    in_names, out_names, out_avals, zero_outs = [], [], [], []
    for alloc in nc.m.functions[0].allocations:
        if not isinstance(alloc, mybir.MemoryLocationSet):
            continue
        nm = alloc.memorylocations[0].name
        if alloc.kind == "ExternalInput":
            if nm != partition_name:
                in_names.append(nm)
        elif alloc.kind == "ExternalOutput":
            out_names.append(nm)
            out_avals.append(_jax.core.ShapedArray(
                tuple(alloc.tensor_shape), mybir.dt.np(alloc.dtype)))
            zero_outs.append(np.zeros(tuple(alloc.tensor_shape),
                                      mybir.dt.np(alloc.dtype)))
    n_params = len(in_names)
    n_outs = len(out_avals)
    in_names_all = in_names + out_names
    if partition_name is not None:
        in_names_all.append(partition_name)
    donate = tuple(range(n_params, n_params + n_outs))

    def _body(*args):
        operands = list(args)
        if partition_name is not None:
            operands.append(partition_id_tensor())
        return tuple(_bass_exec_p.bind(
            *operands, out_avals=tuple(out_avals),
            in_names=tuple(in_names_all), out_names=tuple(out_names),
            lowering_input_output_aliases=(), sim_require_finite=True,
            sim_require_nnan=True, nc=nc))

    devices = _jax.devices()[:8]
    mesh = _Mesh(np.asarray(devices), ("core",))
    in_specs = (_PartitionSpec("core"),) * (n_params + n_outs)
    out_specs = (_PartitionSpec("core"),) * len(out_names)
    sharded = _jax.jit(
        _shard_map(_body, mesh=mesh, in_specs=in_specs, out_specs=out_specs,
                   check_rep=False),
        donate_argnums=donate, keep_unused=True)
    return (sharded, in_names, out_names, zero_outs)


def kernel(**inputs):
    """Full-input GNN forward on 8 TRN2 NeuronCores; returns [64, 4] f32."""
    sharded, in_names, out_names, zero_outs = _build_and_jit()
    core_maps = prep_inputs(inputs)
    concat_in = [np.concatenate([core_maps[c][nm] for c in range(8)], axis=0)
                 for nm in in_names]
    concat_zero = [np.zeros((8 * z.shape[0], *z.shape[1:]), z.dtype)
                   for z in zero_outs]
    out_arrs = sharded(*concat_in, *concat_zero)
    oi = out_names.index("out")
    full = np.asarray(out_arrs[oi]).reshape(8, 4, G)
    return np.concatenate([full[c].T for c in range(8)], axis=0)


# revision 97
# speedup vs baseline: 1.0314x; 1.0314x over previous
"""GNN (GENConv x2 + TopK pool) Bass/Tile kernel for TRN2, data-parallel over
8 NeuronCores (8 graphs per core).

Edge aggregation uses a fixed-degree main layout: node n owns edge slots
[32n, 32n+32) (ghost-padded, corrected analytically via host-known pad
counts), so the scatter-softmax segment sums become strided windowed
tensor_reduce ops -- no prefix-scan carry chains, no big boundary gathers.
Overflow edges (deg > 32, ~7%) go to a 2048-slot dst-sorted spill region per
graph handled by a small scan + boundary gather.

Per-core layout ("fm" = feature-major packed): edge tensors are
[128 = 16feat x 8graph, 18432 slots]; node tensors [128, 512].
"""

import numpy as np
from contextlib import ExitStack

import concourse.bass as bass
import concourse.bacc as bacc
import concourse.mybir as mybir
import concourse.tile as tile
from concourse import library_config

F32 = mybir.dt.float32
F32R = mybir.dt.float32r
F16 = mybir.dt.float16
F8E4 = mybir.dt.float8e4
I16 = mybir.dt.int16
I8 = mybir.dt.int8

G = 8          # graphs per core
N = 512        # nodes per graph
DEG = 32       # fixed main-slot degree per node
EG = 16384     # edges per graph (input)
EM = N * DEG   # main slots per graph = 16384
ESP = 1536     # spill slots per graph (max observed 1248)
ET = EM + ESP  # total slots = 18432
EF = 16        # edge/node feature dim after encode
XF = 64        # input node feature dim
K = 256        # topk keep
CHUNK = 1024   # slots per chunk
NCH = ET // CHUNK        # 18 chunks (16 main + 2 spill)
NMAIN = EM // CHUNK      # 16
NT = 528       # gather table columns (512 nodes + ghost pad; 16-multiple)
GHOST = 512
NEND = 576     # padded spill end-list length (513 used)
EPS = 1e-7
BIGNEG = 6.0e4
LN32 = float(np.log(DEG))

import os as _os
STOP_STAGE = _os.environ.get("K_STOP_STAGE") or None  # hw bisect hook


# ----------------------------------------------------------------------------
# Host-side preprocessing: full inputs -> per-core named arrays
# ----------------------------------------------------------------------------

def prep_inputs(inputs: dict) -> list[dict]:
    x = np.asarray(inputs["x"], np.float32)            # [B*N, 64]
    ei = np.asarray(inputs["edge_index"])              # [2, E] int64
    ea = np.asarray(inputs["edge_attr"], np.float32)   # [E, 16]
    B = 64
    assert x.shape == (B * N, XF)
    assert ea.shape == (B * EG, EF)

    src_g = (ei[0] % N).astype(np.int64)
    dst_g = (ei[1] % N).astype(np.int64)
    graph_of_edge = (ei[0] // N).astype(np.int64)
    assert np.array_equal(graph_of_edge, np.repeat(np.arange(B), EG)), \
        "edge blocks not per-graph; prep assumes reference setup_inputs layout"
    assert np.array_equal(ei[0] // N, ei[1] // N)

    def lin(name):
        return np.asarray(inputs[name], np.float32)

    W_ne, b_ne = lin("W_ne"), lin("b_ne")
    W_ee, b_ee = lin("W_ee"), lin("b_ee")
    W1a, b1a, g1, be1 = lin("W1a"), lin("b1a"), lin("g1"), lin("be1")
    W1b, b1b = lin("W1b"), lin("b1b")
    W2a, b2a, g2, be2 = lin("W2a"), lin("b2a"), lin("g2"), lin("be2")
    W2b, b2b = lin("W2b"), lin("b2b")
    Wa, ba, Wo, bo = lin("Wa"), lin("ba"), lin("Wo"), lin("bo")
    w_pool = lin("w_pool")
    wp = w_pool / np.linalg.norm(w_pool)
    t1 = np.float32(inputs["t1"])
    t2 = np.float32(inputs["t2"])

    # centering fold: LN(y) uses yC = y - mean(y) = u @ (W C) + b C
    C32 = np.eye(32, dtype=np.float32) - 1.0 / 32.0
    W1aC = W1a @ C32
    b1aC = b1a @ C32
    W2aC = W2a @ C32
    b2aC = b2a @ C32

    cst = {}
    wne = np.zeros((128, 4 * 128), np.float32)
    for p in range(4):
        for a in range(2):
            gg = 2 * p + a
            wne[64 * a:64 * a + XF, 128 * p + 16 * gg:128 * p + 16 * gg + EF] = W_ne
    cst["wne_stat"] = wne.astype(np.float16)
    cst["bne_vec"] = np.tile(b_ne, G)[:, None].astype(np.float32)
    wee = np.zeros((128, 128), np.float32)
    for g in range(G):
        wee[16 * g:16 * g + EF, 16 * g:16 * g + EF] = W_ee
    cst["wee_stat"] = wee.astype(np.float16)
    cst["bee_vec"] = np.tile(b_ee, G)[:, None].astype(np.float32)
    cst["mln32_vec"] = np.full((128, 1), -LN32, np.float32)
    cst["identT"] = np.eye(128, dtype=np.float32)
    cst["ident"] = np.eye(128, dtype=np.float32)
    cst["t1vec"] = np.full((128, 1), t1, np.float32)
    cst["t2vec"] = np.full((128, 1), t2, np.float32)
    w1a = np.zeros((64, 128), np.float32)
    for gg in range(4):
        w1a[16 * gg:16 * gg + 16, 32 * gg:32 * gg + 32] = W1aC
    cst["w1a_stat"] = np.vstack([w1a, w1a])
    cst["b1a_vec"] = np.tile(b1aC, 4)[:, None].astype(np.float32)
    ones32 = np.zeros((128, 4), np.float32)
    for gg in range(4):
        ones32[32 * gg:32 * gg + 32, gg] = 1.0 / 32.0
    cst["ones32h_stat"] = ones32.astype(np.float16)
    onesb32 = np.zeros((4, 128), np.float32)
    for gg in range(4):
        onesb32[gg, 32 * gg:32 * gg + 32] = 1.0
    cst["onesb32_stat"] = onesb32
    cst["g1_vec"] = np.tile(g1, 4)[:, None].astype(np.float32)
    cst["be1_vec"] = np.tile(be1, 4)[:, None].astype(np.float32)
    w1b = np.zeros((64, 32), np.float32)
    for gg in range(2):
        w1b[32 * gg:32 * gg + 32, 16 * gg:16 * gg + 16] = W1b
    cst["w1bh_stat"] = np.vstack([w1b, w1b]).astype(np.float16)
    cst["b1b_vec"] = np.tile(b1b, G)[:, None].astype(np.float32)
    wpool = np.zeros((128, 8), np.float32)
    for g in range(G):
        wpool[16 * g:16 * g + EF, g] = wp
    cst["wpool_stat"] = wpool
    ones16b = np.zeros((8, 128), np.float32)
    for g in range(G):
        ones16b[g, 16 * g:16 * g + EF] = 1.0
    cst["ones16b_stat"] = ones16b.astype(np.float16)
    onesel = np.zeros((8, 8 * 128), np.float32)
    for g in range(8):
        onesel[g, 128 * g:128 * (g + 1)] = 1.0
    cst["onesel_stat"] = onesel
    w2a = np.zeros((64, 128), np.float32)
    for gg in range(4):
        w2a[16 * gg:16 * gg + 16, 32 * gg:32 * gg + 32] = W2aC
    cst["w2a_stat"] = np.vstack([w2a, w2a])
    cst["b2a_vec"] = np.tile(b2aC, 4)[:, None].astype(np.float32)
    cst["g2_vec"] = np.tile(g2, 4)[:, None].astype(np.float32)
    cst["be2_vec"] = np.tile(be2, 4)[:, None].astype(np.float32)
    w2b = np.zeros((64, 64), np.float32)
    for gg in range(2):
        w2b[32 * gg:32 * gg + 32, 32 * gg:32 * gg + 32] = W2b
    cst["w2bh_stat"] = np.vstack([w2b, w2b]).astype(np.float16)
    cst["b2b_vec"] = np.tile(b2b, 4)[:, None].astype(np.float32)
    mbA = np.zeros((8, 128), np.float32)
    mbB = np.zeros((8, 128), np.float32)
    for g in range(4):
        mbA[g, 32 * g:32 * g + 32] = 1.0
        mbB[g + 4, 32 * g:32 * g + 32] = 1.0
    cst["maskbc_statA"] = mbA.astype(np.float16)
    cst["maskbc_statB"] = mbB.astype(np.float16)
    selk = np.zeros((128, 4 * 32), np.float32)
    for gg in range(4):
        selk[32 * gg:32 * gg + 32, 32 * gg:32 * gg + 32] = np.eye(32) / K
    cst["selk_stat"] = selk
    d8 = np.zeros((8, 64), np.float32)
    for g in range(G):
        d8[g, 9 * g] = 1.0   # spreads transpose(mask) onto per-graph columns
    cst["d8_stat"] = d8
    cst["wa_stat"] = Wa.astype(np.float32)
    cst["ba_vec"] = ba[:, None].astype(np.float32)
    cst["wo_stat"] = Wo.astype(np.float32)
    cst["bo2_vec"] = (-2.0 * bo)[:, None].astype(np.float32)
    cst["lneps_vec"] = np.full((4, 1), 1e-5, np.float32)

    core_maps = []
    for core in range(8):
        m = dict(cst)
        gsl = slice(core * G, (core + 1) * G)
        xt = np.zeros((128, 4 * 512), np.float32)
        xs = x.reshape(B, N, XF)[gsl]
        for p in range(4):
            for a in range(2):
                xt[64 * a:64 * a + XF, 512 * p:512 * (p + 1)] = xs[2 * p + a].T
        m["xT"] = xt.astype(np.float16)

        attrT = np.zeros((128, ET), np.float16)
        srcidx = np.zeros((128, ET // 16), np.int16)
        spendidx = np.zeros((128, NEND // 16), np.int16)
        padcnt = np.zeros((128, N), np.float16)
        amat = np.zeros((128, 32 * 512), np.float16)
        for gl in range(G):
            gid = core * G + gl
            s_l = src_g[gid * EG:(gid + 1) * EG]
            d_l = dst_g[gid * EG:(gid + 1) * EG]
            order = np.argsort(d_l, kind="stable")
            ds = d_l[order]
            ss = s_l[order]
            ats = ea[gid * EG:(gid + 1) * EG][order]     # [EG, 16] dst-sorted
            counts = np.bincount(ds, minlength=N)
            starts = np.zeros(N + 1, np.int64)
            np.cumsum(counts, out=starts[1:])
            # main: node n gets its first min(deg,32) edges at slots 32n+o
            j = np.arange(EM)
            nn = j // DEG
            oo = j % DEG
            msk = oo < np.minimum(counts[nn], DEG)
            pos = starts[nn] + oo
            srcf = np.full(ET, GHOST, np.int64)
            attrf = np.zeros((ET, EF), np.float32)
            srcf[j[msk]] = ss[pos[msk]]
            attrf[j[msk]] = ats[pos[msk]]
            # spill: rank within dst-run >= 32 (already dst-sorted)
            r = np.arange(EG) - starts[ds]
            spm = r >= DEG
            nsp = int(spm.sum())
            assert nsp <= ESP, f"spill overflow: {nsp} > {ESP}"
            srcf[EM:EM + nsp] = ss[spm]
            attrf[EM:EM + nsp] = ats[spm]
            spd = ds[spm]
            e_sp = np.searchsorted(spd, np.arange(N), side="right")
            elist = np.zeros(NEND, np.int16)
            elist[1:N + 1] = e_sp.astype(np.int16)

            attrT[16 * gl:16 * gl + EF, :] = attrf.T.astype(np.float16)
            srcidx[16 * gl:16 * gl + 16, :] = \
                srcf.astype(np.int16).reshape(ET // 16, 16).T
            spendidx[16 * gl:16 * gl + 16, :] = \
                elist.reshape(NEND // 16, 16).T
            padcnt[16 * gl:16 * gl + 16, :] = \
                np.maximum(0, DEG - counts)[None, :].astype(np.float16)
            # adjacency count matrix A[m, n] = #edges m->n (for the L2
            # masked-src denominator correction via matmul)
            A = np.zeros((N, N), np.int32)
            np.add.at(A, (s_l, d_l), 1)
            assert A.max() <= 2048, A.max()  # f16 integers exact to 2048
            for b in range(4):
                amat[:, (4 * gl + b) * 512:(4 * gl + b + 1) * 512] = \
                    A[128 * b:128 * (b + 1), :].astype(np.float16)
        m["spendidx"] = spendidx
        m["padcnt"] = padcnt
        blob = np.zeros((128, CBLOB_BYTES), np.uint8)
        for name, shape, dt, off in CONST_SPECS:
            arr = m[name]
            bv = arr.view(np.uint8).reshape(arr.shape[0], -1)
            blob[:arr.shape[0], off:off + bv.shape[1]] = bv
        rblob = np.zeros((128, 4 * 128), np.float32)
        rblob[:, 0:128] = m["ident"]
        rblob[:, 128:256] = m["w1a_stat"]
        rblob[:, 256:384] = m["w2a_stat"]
        rblob[0:4, 384:512] = m["onesb32_stat"]
        core_maps.append({"cblob": blob, "rblob": rblob, "attrT": attrT,
                          "xT": m["xT"], "srcidx": srcidx, "amat": amat})
    return core_maps


_CSPEC_RAW = [
    # encode-critical constants first (covered by the first cblob DMA piece)
    ("wne_stat", [128, 4 * 128], F16),
    ("bne_vec", [128, 1], F32),
    ("wee_stat", [128, 128], F16),
    ("bee_vec", [128, 1], F32),
    ("mln32_vec", [128, 1], F32),
    ("t1vec", [128, 1], F32),
    ("t2vec", [128, 1], F32),
    ("spendidx", [128, NEND // 16], I16),
    ("padcnt", [128, N], F16),
    # ---- split point: everything below arrives with the second DMA ----
    ("identT", [128, 128], F32),
    ("b1a_vec", [128, 1], F32),
    ("ones32h_stat", [128, 4], F16),
    ("g1_vec", [128, 1], F32),
    ("be1_vec", [128, 1], F32),
    ("w1bh_stat", [128, 32], F16),
    ("b1b_vec", [128, 1], F32),
    ("wpool_stat", [128, 8], F32),
    ("ones16b_stat", [8, 128], F16),
    ("onesel_stat", [8, 8 * 128], F32),
    ("b2a_vec", [128, 1], F32),
    ("g2_vec", [128, 1], F32),
    ("be2_vec", [128, 1], F32),
    ("w2bh_stat", [128, 64], F16),
    ("b2b_vec", [128, 1], F32),
    ("maskbc_statA", [8, 128], F16),
    ("maskbc_statB", [8, 128], F16),
    ("selk_stat", [128, 4 * 32], F32),
    ("d8_stat", [8, 64], F32),
    ("wa_stat", [32, 128], F32),
    ("ba_vec", [128, 1], F32),
    ("wo_stat", [128, 4], F32),
    ("bo2_vec", [4, 1], F32),
    ("lneps_vec", [4, 1], F32),
]

_SPLIT_AFTER = "padcnt"


def _mk_const_specs():
    specs = []
    off = 0
    for nm, shape, dt in _CSPEC_RAW:
        nbytes = int(np.prod(shape[1:])) * mybir.dt.size(dt)
        specs.append((nm, shape, dt, off))
        off += (nbytes + 127) // 128 * 128
    return specs, off

CONST_SPECS, CBLOB_BYTES = _mk_const_specs()

INPUT_SPECS = [
    ("cblob", [128, CBLOB_BYTES], mybir.dt.uint8),
    ("rblob", [128, 4 * 128], F32R),   # ident | w1a | w2a | onesb32(rows 0:4)
    ("attrT", [128, ET], F16),
    ("xT", [128, 4 * 512], F16),
    ("srcidx", [128, ET // 16], I16),
    ("amat", [128, 32 * 512], F16),
]


# ----------------------------------------------------------------------------
# Device graph
# ----------------------------------------------------------------------------

def build_nc(debug_keys=(), n_rep=1):
    nc = bacc.Bacc(None, target_bir_lowering=False, debug=False)
    A = {}
    for name, shape, dt in INPUT_SPECS:
        A[name] = nc.declare_dram_parameter(name, shape, dt, isOutput=False)[:]
    out_ext = nc.declare_dram_parameter("out", [4, G], F32, isOutput=True)[:]
    dbg_ext = {}
    dbg_shapes = {
        "hT": ((128, N), F32), "mc0": ((128, CHUNK), F16),
        "ev0": ((128, CHUNK), F16), "evm0": ((128, CHUNK), F16),
        "sums0": ((128, N), F16), "sums1": ((128, N), F16),
        "spev0": ((128, ESP), F16), "Ssp0": ((128, ESP + 1), F32),
        "Dsp0": ((128, N), F32), "Dsp1": ((128, N), F32),
        "aggr1": ((128, N), F32), "u1": ((128, N), F32),
        "h1": ((128, N), F32),
        "score": ((8, N), F32), "rk": ((128, 32), F32),
        "mask": ((128, 32), F32),
        "hq": ((128, N), F32), "hp": ((128, N), F32),
        "aggr2": ((128, N), F32), "h2a": ((128, N), F16),
        "h2b": ((128, N), F16), "pooled": ((32, 8), F32),
        "y1sA": ((128, N), F16), "r1A": ((128, N), F16),
    }
    for key in debug_keys:
        shape, dt = dbg_shapes[key]
        dbg_ext[key] = nc.declare_dram_parameter(
            "dbg_" + key, list(shape), dt, isOutput=True)[:]

    with tile.TileContext(nc) as tc, ExitStack() as ctx:
        consts = ctx.enter_context(tc.tile_pool(name="consts", bufs=1))
        nodep = ctx.enter_context(tc.tile_pool(name="nodep", bufs=1))
        work = ctx.enter_context(tc.tile_pool(name="work", bufs=2))
        psum = ctx.enter_context(tc.tile_pool(name="psum", bufs=2, space="PSUM"))
        psum1 = ctx.enter_context(tc.tile_pool(name="psum1", bufs=1, space="PSUM"))
        psume = ctx.enter_context(tc.tile_pool(name="psume", bufs=1, space="PSUM"))
        psumw = ctx.enter_context(tc.tile_pool(name="psumw", bufs=1, space="PSUM"))

        nc.gpsimd.load_library(library_config.ap_gather)

        # ---- DMAs: xT first (encode-critical), then cblob, srcidx, attr ----
        xts = consts.tile([128, 4 * 512], F16, name="xT_sb")
        nc.sync.dma_start(out=xts, in_=A["xT"])
        cb = consts.tile([128, CBLOB_BYTES], mybir.dt.uint8, name="cblob_sb")
        _split = next(off for nm, _s, _d, off in CONST_SPECS
                      if nm == "identT")
        nc.sync.dma_start(out=cb[:, :_split], in_=A["cblob"][:, :_split])
        srcidx_sb = consts.tile([128, ET // 16], I16, name="srcidx_sb")
        nc.sync.dma_start(out=srcidx_sb, in_=A["srcidx"])
        attr_all = consts.tile([128, ET], F16, name="attr_sb")
        # pieces in chunk-consumption order: main[0:2048], spill, rest
        nc.sync.dma_start(out=attr_all[:, 0:2048], in_=A["attrT"][:, 0:2048])
        rb_sb = consts.tile([128, 4 * 128], F32R, name="rblob_sb")
        nc.sync.dma_start(out=rb_sb, in_=A["rblob"])
        nc.sync.dma_start(out=attr_all[:, EM:], in_=A["attrT"][:, EM:])
        for lo, hi in ((2048, 8192), (8192, 12288), (12288, EM)):
            nc.sync.dma_start(out=attr_all[:, lo:hi], in_=A["attrT"][:, lo:hi])
        nc.sync.dma_start(out=cb[:, _split:], in_=A["cblob"][:, _split:])
        amat_sb = consts.tile([128, 16 * 512], F16, name="amat_sb")
        nc.sync.dma_start(out=amat_sb, in_=A["amat"][:, :16 * 512])

        C = {"srcidx": srcidx_sb, "attr": attr_all, "amat": amat_sb,
             "amat_dram": A["amat"],
             "ident": rb_sb[:, 0:128], "w1a_stat": rb_sb[:, 128:256],
             "w2a_stat": rb_sb[:, 256:384],
             "onesb32_stat": rb_sb[0:4, 384:512]}
        for name, shape, dt, off in CONST_SPECS:
            nbytes = int(np.prod(shape[1:])) * mybir.dt.size(dt)
            ap = cb[:shape[0], off:off + nbytes].bitcast(dt)
            if len(shape) == 3:
                ap = ap.rearrange("p (a b) -> p a b", b=shape[2])
            C[name] = ap

        dbg_keys_set = set(debug_keys)

        def dbg(key, ap):
            if key in dbg_keys_set:
                nc.sync.dma_start(out=dbg_ext[key], in_=ap)

        for rep in range(n_rep):
            run_once(nc, tc, A, C, xts, out_ext, dbg, nodep, work,
                     psum, psum1, psume, psumw, rep)
    nc.compile()
    return nc


def run_once(nc, tc, A, C, xts, out_ext, dbg, nodep, work, psum, psum1,
             psume, psumw, rep):
    V = nc.vector
    S = nc.scalar
    T = nc.tensor
    Alu = mybir.AluOpType
    Act = mybir.ActivationFunctionType

    # ================= node encode: hT = x @ W_ne + b_ne =================
    hraw = psume.tile([128, N], F32, name="hraw", tag="e512")
    for p in range(4):
        T.matmul(hraw, C["wne_stat"][:, 128 * p:128 * (p + 1)],
                 xts[:, 512 * p:512 * (p + 1)],
                 start=(p == 0), stop=(p == 3))
    hT = nodep.tile([128, NT, 1], F32, name=f"hT_{rep}")
    S.activation(hT[:, :N, 0], hraw, Act.Identity, bias=C["bne_vec"], scale=1.0)
    S.activation(hT[:, N:, 0], hraw[:, 0:NT - N], Act.Copy, bias=0.0, scale=0.0)
    dbg("hT", hT[:, :N, 0])

    def stop_dma(ap):
        nc.sync.dma_start(out=out_ext, in_=ap)

    if STOP_STAGE == "enc":
        return stop_dma(hT[0:4, 0:G, 0].bitcast(F32))

    # ================= shared edge-phase machinery =================
    def edge_layer(layer, table, tvec, bee, mm=None, dbg_on=False,
                   inject=None):
        """Returns aggr tile [128, N] f32 (softmax-weighted mean + pad fix)."""
        sums = nodep.tile([128, 2, N], F32, name=f"sums_{layer}_{rep}",
                          tag="sums")
        spev = nodep.tile([128, 2, ESP], F16, name=f"spev_{layer}_{rep}",
                          tag="spev")
        # ev for ghost-pad slots: mc_pad = relu(bee); ev_pad = exp(t*mc - ln32)
        beer = nodep.tile([128, 1], F32, name=f"beer_{layer}_{rep}", tag="beer")
        S.activation(beer, bee, Act.Relu, bias=0.0, scale=1.0)
        evpad = nodep.tile([128, 1], F32, name=f"evpad_{layer}_{rep}",
                           tag="evpad")
        S.activation(evpad, beer, Act.Exp, bias=C["mln32_vec"], scale=tvec)
        evmpad = nodep.tile([128, 1], F32, name=f"evmpad_{layer}_{rep}",
                            tag="evmpad")
        V.tensor_tensor(out=evmpad, in0=evpad, in1=beer, op=Alu.mult)

        # 2 main chunks prime the pipeline, then spill (so its scan +
        # boundary-gather tail overlaps the remaining main chunks)
        E = (sums, None, evpad, evmpad)
        for cc in [0, 1, NMAIN, NMAIN + 1] + list(range(2, NMAIN)):
            if inject is not None and cc == inject[0]:
                E = (sums, Dsp, evpad, evmpad)
                inject[1](E)
            base = cc * CHUNK
            csz = CHUNK if cc != NMAIN + 1 else ESP - CHUNK
            hsrc = work.tile([128, CHUNK, 1], F32, name="hsrc", tag="hsrc")
            j0 = base // 16
            nc.gpsimd.ap_gather(
                hsrc[:, :csz, :], table, C["srcidx"][:, j0:j0 + csz // 16],
                channels=128, num_elems=NT, d=1, num_idxs=csz)
            if STOP_STAGE == "pipe1a" and layer == 0:
                return None
            zc = psum.tile([128, CHUNK], F32, name="zc", tag="zc")
            for s in range(csz // 512):
                sl = slice(512 * s, 512 * (s + 1))
                T.matmul(zc[:, sl], C["wee_stat"],
                         C["attr"][:, base + 512 * s:base + 512 * (s + 1)],
                         start=True, stop=False, skip_group_check=True)
                T.matmul(zc[:, sl], C["ident"].bitcast(F32), hsrc[:, sl, 0],
                         start=False, stop=True, skip_group_check=True)
            mc = work.tile([128, CHUNK], F16, name="mc", tag="mc")
            S.activation(mc[:, :csz], zc[:, :csz], Act.Relu, bias=bee,
                         scale=1.0)
            if STOP_STAGE == "pipe1b" and layer == 0:
                return None
            if cc < NMAIN:
                evcc = work.tile([128, 2, CHUNK], F16, name="evcc", tag="evcc")
                ev_t, evm_t = evcc[:, 0, :], evcc[:, 1, :]
            else:
                sp = cc - NMAIN
                spsl = slice(CHUNK * sp, CHUNK * sp + csz)
                ev_t, evm_t = spev[:, 0, spsl], spev[:, 1, spsl]
            mc = mc[:, :csz]
            # L2 masked-src edges (hq[src] = -2e9): mc = 0 exactly, so they
            # add 0 to the numerator and exactly 1/32 to the denominator;
            # the denominator excess is removed analytically via `mm`
            # (adjacency matmul) instead of a per-edge mask multiply.
            S.activation(ev_t, mc, Act.Exp, bias=C["mln32_vec"], scale=tvec)
            V.tensor_tensor(out=evm_t, in0=ev_t, in1=mc, op=Alu.mult)
            if STOP_STAGE == "pipe1" and layer == 0:
                return None
            if cc < NMAIN:
                # windowed segment-sum via pairwise folds (f16 2x DVE mode)
                # + small f32 reduce: ~1550ns vs 2254 for one big 1x reduce
                f1 = work.tile([128, 2, 32, 16], F16, name="f1", tag="f1")
                ev4 = evcc[:, :, :].rearrange("p a (b c) -> p a b c", c=DEG)
                V.tensor_tensor(out=f1, in0=ev4[:, :, :, 0:16],
                                in1=ev4[:, :, :, 16:32], op=Alu.add)
                f2 = work.tile([128, 2, 32, 8], F16, name="f2", tag="f2")
                V.tensor_tensor(out=f2, in0=f1[:, :, :, 0:8],
                                in1=f1[:, :, :, 8:16], op=Alu.add)
                f3 = work.tile([128, 2, 32, 4], F16, name="f3", tag="f3")
                V.tensor_tensor(out=f3, in0=f2[:, :, :, 0:4],
                                in1=f2[:, :, :, 4:8], op=Alu.add)
                V.tensor_reduce(
                    out=sums[:, :, DEG * cc:DEG * (cc + 1)],
                    in_=f3, axis=mybir.AxisListType.X, op=Alu.add)
            if dbg_on and cc == 0:
                dbg("mc0", mc)
                dbg("ev0", ev_t)
                dbg("evm0", evm_t)
            if cc == NMAIN + 1:
                # spill scans + boundary gather (overlap with main chunks)
                Dsp = []
                for ti in range(2):
                    Ssp = nodep.tile([128, ESP + 1, 1], F32,
                                     name=f"Ssp{ti}_{layer}_{rep}",
                                     tag=f"Ssp{ti}")
                    V.memset(Ssp[:, 0:1, 0], 0.0)
                    V.tensor_tensor_scan(
                        out=Ssp[:, 1:, 0], data0=spev[:, ti, :],
                        data1=spev[:, ti, :], initial=0.0,
                        op0=Alu.add, op1=Alu.bypass)
                    gsp = nodep.tile([128, NEND, 1], F32,
                                     name=f"gsp{ti}_{layer}_{rep}",
                                     tag=f"gsp{ti}")
                    nc.gpsimd.ap_gather(
                        gsp, Ssp, C["spendidx"],
                        channels=128, num_elems=ESP + 1, d=1, num_idxs=NEND)
                    d = nodep.tile([128, N], F32, name=f"Dsp{ti}_{layer}_{rep}",
                                   tag=f"Dsp{ti}")
                    V.tensor_tensor(out=d, in0=gsp[:, 1:N + 1, 0],
                                    in1=gsp[:, 0:N, 0], op=Alu.subtract)
                    Dsp.append(d)
                if dbg_on:
                    dbg("spev0", spev[:, 0, :])
                    dbg("Ssp0", Ssp[:, :, 0])
                    dbg("Dsp0", Dsp[0])
                    dbg("Dsp1", Dsp[1])

        if dbg_on:
            dbg("sums0", sums[:, 0, :])
            dbg("sums1", sums[:, 1, :])
        return (sums, Dsp, evpad, evmpad)

    def aggr_half(E, hs, base_in, mm, layer, hi):
        """Softmax-mean for node slice hs; returns u = base + aggr + EPS."""
        sums, Dsp, evpad, evmpad = E
        HN = hs.stop - hs.start
        neg = []
        for ti, padv in ((0, evpad), (1, evmpad)):
            tot = nodep.tile([128, HN], F32, name=f"tot{ti}_{layer}{hi}_{rep}",
                             tag=f"tot{ti}")
            V.tensor_tensor(out=tot, in0=sums[:, ti, hs], in1=Dsp[ti][:, hs],
                            op=Alu.add)
            if ti == 0 and mm is not None:
                # tot += (mask-1)^T A / 32 (mm pre-scaled by 1/32)
                mmb = psume.tile([128, HN], F32, name="mmb", tag="e512")
                T.matmul(mmb, C["ones16b_stat"], mm[:, hs],
                         start=True, stop=True)
                tot2 = nodep.tile([128, HN], F32,
                                  name=f"tot2_{layer}{hi}_{rep}", tag="tot2")
                V.tensor_tensor(out=tot2, in0=mmb, in1=tot, op=Alu.add)
                tot = tot2
            ng = nodep.tile([128, HN], F32, name=f"ng{ti}_{layer}{hi}_{rep}",
                            tag=f"ng{ti}")
            V.scalar_tensor_tensor(out=ng, in0=C["padcnt"][:, hs], scalar=padv,
                                   in1=tot, op0=Alu.mult, op1=Alu.subtract)
            neg.append(ng)
        dm = nodep.tile([128, HN], F32, name=f"dm_{layer}{hi}_{rep}", tag="dm")
        V.tensor_scalar(dm, neg[0], -1e-16, None, Alu.min)
        rec = nodep.tile([128, HN], F32, name=f"rec_{layer}{hi}_{rep}",
                         tag="rec")
        V.reciprocal(rec, dm)
        ag = nodep.tile([128, HN], F32, name=f"ag_{layer}{hi}_{rep}", tag="ag")
        V.tensor_tensor(out=ag, in0=neg[1], in1=rec, op=Alu.mult)
        u = nodep.tile([128, HN], F32R, name=f"u_{layer}{hi}_{rep}", tag="u")
        V.scalar_tensor_tensor(out=u, in0=ag, scalar=EPS, in1=base_in[:, hs],
                               op0=Alu.add, op1=Alu.add)
        return u

    def mlp_half(uin, hs, wa_stat, ba_vec, gvec, bevec, wb_stat, layer, hi):
        HN = hs.stop - hs.start
        N2 = 2 * HN
        y1p = psumw.tile([128, 2, HN], F32, name=f"y1p_{layer}{hi}",
                         tag="wide")
        for half in range(2):
            T.matmul(y1p[:, half, :],
                     wa_stat[64 * half:64 * half + 64, :],
                     uin[64 * half:64 * half + 64, :],
                     start=True, stop=True)
        y1pf = y1p.rearrange("p a b -> p (a b)")
        y1s = nodep.tile([128, N2], F16, name=f"y1s_{layer}{hi}_{rep}",
                         tag="y1s")
        S.activation(y1s, y1pf, Act.Identity, bias=ba_vec, scale=1.0)
        sq = nodep.tile([128, N2], F16, name=f"sq_{layer}{hi}_{rep}", tag="sq")
        V.tensor_tensor(out=sq, in0=y1s, in1=y1s, op=Alu.mult)
        vp = psumw.tile([4, 2, HN], F32, name=f"vp_{layer}{hi}", tag="wide")
        for half in range(2):
            T.matmul(vp[:, half, :], C["ones32h_stat"],
                     sq[:, half * HN:half * HN + HN], start=True, stop=True)
        lnv = nodep.tile([4, N2], F32, name=f"lnv_{layer}{hi}_{rep}",
                         tag="st4", bufs=2)
        S.activation(lnv, vp.rearrange("p a b -> p (a b)"), Act.Ln,
                     bias=C["lneps_vec"], scale=1.0)
        rstd = nodep.tile([4, N2], F32R, name=f"rstd_{layer}{hi}_{rep}",
                          tag="st4", bufs=2)
        S.activation(rstd, lnv, Act.Exp, bias=0.0, scale=-0.5)
        rb = psumw.tile([128, 2, HN], F32, name=f"rb_{layer}{hi}", tag="wide")
        for half in range(2):
            T.matmul(rb[:, half, :], C["onesb32_stat"],
                     rstd[:, half * HN:half * HN + HN],
                     start=True, stop=True)
        vnorm = nodep.tile([128, N2], F32, name=f"vn_{layer}{hi}_{rep}",
                           tag="vn")
        V.tensor_tensor(out=vnorm, in0=y1s,
                        in1=rb.rearrange("p a b -> p (a b)"), op=Alu.mult)
        r1 = nodep.tile([128, N2], F16, name=f"r1_{layer}{hi}_{rep}", tag="r1")
        S.activation(r1, vnorm, Act.Relu, bias=bevec, scale=gvec)
        M = wb_stat.shape[1]
        outs = []
        for half in range(2):
            for q in range(2):
                yq = psum.tile([M, HN], F32, name=f"yq{half}{q}", tag="zc")
                T.matmul(yq, wb_stat[64 * q:64 * q + 64, :],
                         r1[64 * q:64 * q + 64, half * HN:half * HN + HN],
                         start=True, stop=True)
                outs.append(yq)
        return outs

    HALVES = (slice(0, N),)

    # ================= Layer 1 =================
    h1 = nodep.tile([128, N], F32, name=f"h1_{rep}", tag="h1")

    def half_pipe(layer, base_in, mm, wa, ba, gv, bev, wb, hout):
        def emit(E, hi, hs):
            u = aggr_half(E, hs, base_in, mm, layer, hi)
            y2q = mlp_half(u, hs, wa, ba, gv, bev, wb, layer, hi)
            for q in range(4):
                if layer == 0:
                    S.activation(hout[32 * q:32 * q + 32, hs], y2q[q],
                                 Act.Relu,
                                 bias=C["b1b_vec"][32 * q:32 * q + 32, :],
                                 scale=1.0)
                else:
                    sl, part = q // 2, q % 2
                    S.activation(hout[sl][64 * part:64 * part + 64, hs],
                                 y2q[q], Act.Relu,
                                 bias=C["b2b_vec"][64 * part:64 * part + 64, :],
                                 scale=1.0)
        return emit

    emit1 = half_pipe(0, hT[:, :N, 0], None, C["w1a_stat"], C["b1a_vec"],
                      C["g1_vec"], C["be1_vec"], C["w1bh_stat"], h1)
    E1 = edge_layer(0, hT, C["t1vec"], C["bee_vec"], dbg_on=True)
    if STOP_STAGE in ("pipe1", "pipe1a", "pipe1b"):
        return None
    emit1(E1, 0, HALVES[0])
    dbg("h1", h1)
    if STOP_STAGE == "mlp1":
        return stop_dma(h1[0:4, 0:G])

    # ================= score / topk mask / gates =================
    scp = psume.tile([8, N], F32, name="scp", tag="e512")
    T.matmul(scp, C["wpool_stat"], h1, start=True, stop=True)
    scs = nodep.tile([8, N], F32, name=f"scs_{rep}", tag="scs")
    S.activation(scs, scp, Act.Copy, bias=0.0, scale=1.0)
    dbg("score", scs)
    snode = nodep.tile([128, 4, 8], F32, name=f"snode_{rep}", tag="snode")
    for t in range(4):
        tp = psum1.tile([128, 8], F32, name="tp", tag="small")
        T.transpose(tp, scs[:, 128 * t:128 * (t + 1)], C["identT"][0:8, 0:8])
        S.activation(snode[:, t, :], tp, Act.Copy, bias=0.0, scale=1.0)
    sneg = nodep.tile([128, 4, 8], F32, name=f"sneg_{rep}", tag="sneg")
    V.tensor_scalar(sneg, snode, -1.0, None, Alu.mult)
    # rank: graphs 0-3 on Act (sign-sum), graphs 4-7 on DVE (is_gt count)
    rk = nodep.tile([128, 4, 8], F32, name=f"rk_{rep}", tag="rk")
    # interleave Act-half (g<4) and DVE-half (g>=4) so both engines run
    # concurrently despite the 2-deep sb psum ring
    for g in (0, 4, 1, 5, 2, 6, 3, 7):
        sb = psum.tile([128, N], F32, name="sb", tag="zc")
        T.matmul(sb, C["onesel_stat"][:, 128 * g:128 * (g + 1)],
                 scs, start=True, stop=True)
        if g < 4:
            for t in range(4):
                sga = work.tile([128, N], F16, name="sga", tag="sga")
                S.activation(sga, sb, Act.Sign, bias=sneg[:, t, g:g + 1],
                             scale=1.0, accum_out=rk[:, t, g:g + 1])
        else:
            for t in range(4):
                sgv = work.tile([128, N], F16, name="sgv", tag="sgv")
                V.tensor_scalar(sgv, sb, snode[:, t, g:g + 1], 0.0,
                                Alu.is_gt, Alu.add,
                                accum_out=rk[:, t, g:g + 1])
    dbg("rk", rk.rearrange("p a b -> p (a b)"))
    if STOP_STAGE == "rank":
        return stop_dma(rk[0:4, 0, 0:G])
    mask01 = nodep.tile([128, 4, 8], F32, name=f"mask01_{rep}", tag="mask01")
    V.tensor_scalar(mask01[:, :, 0:4], rk[:, :, 0:4], -1.0, None, Alu.is_le)
    V.tensor_scalar(mask01[:, :, 4:8], rk[:, :, 4:8], float(K) - 0.5, None,
                    Alu.is_le)
    dbg("mask", mask01.rearrange("p a b -> p (a b)"))
    ex = nodep.tile([128, 4, 8], F32, name=f"ex_{rep}", tag="ex")
    S.activation(ex, snode, Act.Exp, bias=0.0, scale=-2.0)
    exd = nodep.tile([128, 4, 8], F32, name=f"exd_{rep}", tag="exd")
    V.tensor_scalar(exd, ex, 1.0, None, Alu.add)
    exr = nodep.tile([128, 4, 8], F32, name=f"exr_{rep}", tag="exr")
    V.reciprocal(exr, exd)
    th = nodep.tile([128, 4, 8], F32, name=f"th_{rep}", tag="th")
    V.tensor_scalar(th, exr, 2.0, -1.0, Alu.mult, Alu.add)
    gate = nodep.tile([128, 4, 8], F32, name=f"gate_{rep}", tag="gate")
    V.tensor_tensor(out=gate, in0=th, in1=mask01, op=Alu.mult)
    gq = nodep.tile([128, 4, 8], F32, name=f"gq_{rep}", tag="gq")
    V.tensor_scalar(gq, mask01, -1.0, BIGNEG, Alu.add, Alu.mult)
    gfm = nodep.tile([8, N], F16, name=f"gfm_{rep}", tag="gfm")
    qfm = nodep.tile([8, N], F16, name=f"qfm_{rep}", tag="qfm")
    mfm = nodep.tile([8, N], F16, name=f"mfm_{rep}", tag="mfm")
    for t in range(4):
        tstack = work.tile([128, 96], F32, name="tstack", tag="tstack")
        V.tensor_copy(out=tstack[:, 0:8], in_=gate[:, t, :])
        V.tensor_copy(out=tstack[:, 32:40], in_=gq[:, t, :])
        V.tensor_copy(out=tstack[:, 64:72], in_=mask01[:, t, :])
        tq = psum1.tile([96, 128], F32, name="tq", tag="small")
        T.transpose(tq, tstack, C["identT"])
        S.activation(gfm[:, 128 * t:128 * (t + 1)], tq[0:8, :], Act.Copy,
                     bias=0.0, scale=1.0)
        S.activation(qfm[:, 128 * t:128 * (t + 1)], tq[32:40, :], Act.Copy,
                     bias=0.0, scale=1.0)
        S.activation(mfm[:, 128 * t:128 * (t + 1)], tq[64:72, :], Act.Copy,
                     bias=0.0, scale=1.0)
    # masked-src edge counts: mmg[g, n] = sum_m (mask_g[m]-1) * A_g[m, n];
    # per (g, block) lhsT is [128, 8] with only column g nonzero, so all 32
    # matmuls accumulate into one [8, N] psum (zero rows elsewhere)
    mfm1 = nodep.tile([8, N], F32, name=f"mfm1_{rep}", tag="mfm1")
    V.tensor_scalar(mfm1, mfm, -1.0, None, Alu.add)
    zsb = nodep.tile([128, 4, 64], F16, name=f"zsb_{rep}", tag="zsb")
    for b in range(4):
        zp = psum1.tile([128, 64], F32, name="zp", tag="small")
        T.matmul(zp, mfm1[:, 128 * b:128 * (b + 1)], C["d8_stat"],
                 start=True, stop=True)
        S.activation(zsb[:, b, :], zp, Act.Copy, bias=0.0, scale=1.0)
    mmg_ps = psumw.tile([8, N], F32, name="mmg_ps", tag="wide")
    for g in range(G):
        if g == 4:   # second half of A overwrites the buffer (WAR-tracked)
            nc.sync.dma_start(out=C["amat"], in_=C["amat_dram"][:, 16 * 512:])
        for b in range(4):
            T.matmul(mmg_ps, zsb[:, b, 8 * g:8 * g + 8],
                     C["amat"][:, ((g % 4) * 4 + b) * 512:
                               ((g % 4) * 4 + b + 1) * 512],
                     start=(g == 0 and b == 0), stop=(g == 7 and b == 3))
    mmg_s = nodep.tile([8, N], F16, name=f"mmg_s_{rep}", tag="mmg_s")
    S.activation(mmg_s, mmg_ps, Act.Copy, bias=0.0, scale=1.0 / DEG)

    gb = psume.tile([128, N], F32, name="gb", tag="e512")
    T.matmul(gb, C["ones16b_stat"], gfm, start=True, stop=True)
    hp = nodep.tile([128, N], F32, name=f"hp_{rep}", tag="hp")
    V.tensor_tensor(out=hp, in0=h1, in1=gb, op=Alu.mult)
    dbg("hp", hp)
    qb = psume.tile([128, N], F32, name="qb", tag="e512")
    T.matmul(qb, C["ones16b_stat"], qfm, start=True, stop=True)
    hq = nodep.tile([128, NT, 1], F32, name=f"hq_{rep}", tag="hq")
    V.tensor_tensor(out=hq[:, :N, 0], in0=hp, in1=qb, op=Alu.add)
    S.activation(hq[:, N:, 0], qb[:, 0:NT - N], Act.Copy, bias=0.0, scale=0.0)
    dbg("hq", hq[:, :N, 0])
    if STOP_STAGE == "hq":
        return stop_dma(hq[0:4, 0:G, 0].bitcast(F32))

    # ================= Layer 2 =================
    h2 = [nodep.tile([128, N], F16, name=f"h2{sl}_{rep}", tag=f"h2{sl}")
          for sl in range(2)]
    emit2 = half_pipe(1, hp, mmg_s, C["w2a_stat"], C["b2a_vec"],
                      C["g2_vec"], C["be2_vec"], C["w2bh_stat"], h2)
    E2 = edge_layer(1, hq, C["t2vec"], C["bee_vec"], mm=mmg_s)
    if STOP_STAGE == "l2agg":
        return stop_dma(mmg_s[0:4, 0:G])
    emit2(E2, 0, HALVES[0])
    dbg("h2a", h2[0])
    dbg("h2b", h2[1])

    # ================= pooling + head =================
    pooled = []
    for sl, statname in ((0, "maskbc_statA"), (1, "maskbc_statB")):
        mb2 = psume.tile([128, N], F32, name=f"mbp{sl}", tag="e512")
        T.matmul(mb2, C[statname], mfm, start=True, stop=True)
        mbh = nodep.tile([128, N], F16, name=f"mbh{sl}_{rep}", tag=f"mbh{sl}")
        S.activation(mbh, mb2, Act.Copy, bias=0.0, scale=1.0)
        pl = nodep.tile([128, 1], F32, name=f"pl{sl}_{rep}", tag=f"pl{sl}")
        scratch = work.tile([128, N], F16, name="plscratch", tag="plscratch",
                            bufs=1)
        V.scalar_tensor_tensor(out=scratch, in0=h2[sl], scalar=1.0, in1=mbh,
                               op0=Alu.mult, op1=Alu.mult, accum_out=pl)
        pooled.append(pl)
    P8 = psum1.tile([32, G], F32, name="P8", tag="small")
    for g in range(G):
        sl, gg = g // 4, g % 4
        T.matmul(P8[:, g:g + 1],
                 C["selk_stat"][:, 32 * gg:32 * gg + 32],
                 pooled[sl], start=True, stop=True,
                 skip_group_check=True)
    p8s = nodep.tile([32, G], F32, name=f"p8s_{rep}", tag="p8s")
    S.activation(p8s, P8, Act.Copy, bias=0.0, scale=1.0)
    dbg("pooled", p8s)
    a1p = psume.tile([128, G], F32, name="a1p", tag="e512")
    T.matmul(a1p, C["wa_stat"], p8s, start=True, stop=True)
    a1 = nodep.tile([128, G], F32, name=f"a1_{rep}", tag="a1")
    S.activation(a1, a1p, Act.Relu, bias=C["ba_vec"], scale=1.0)
    op = psum1.tile([4, G], F32, name="op", tag="small")
    T.matmul(op, C["wo_stat"], a1, start=True, stop=True)
    oe = nodep.tile([4, G], F32, name=f"oe_{rep}", tag="oe")
    S.activation(oe, op, Act.Exp, bias=C["bo2_vec"], scale=-2.0)
    od = nodep.tile([4, G], F32, name=f"od_{rep}", tag="od")
    V.tensor_scalar(od, oe, 1.0, None, Alu.add)
    orr = nodep.tile([4, G], F32, name=f"orr_{rep}", tag="orr")
    V.reciprocal(orr, od)
    ot = nodep.tile([4, G], F32, name=f"ot_{rep}", tag="ot")
    V.tensor_scalar(ot, orr, 2.0, -1.0, Alu.mult, Alu.add)
    nc.sync.dma_start(out=out_ext, in_=ot)


# ----------------------------------------------------------------------------
# Self-contained entry point: kernel(**inputs) -> [64, 4] float32
# ----------------------------------------------------------------------------
import jax as _jax
from jax.sharding import Mesh as _Mesh, PartitionSpec as _PartitionSpec
from jax.experimental.shard_map import shard_map as _shard_map

_COMPILED = {}


def _build_and_jit():
    """Re-create the jitted executable on every call: re-executing a loaded
    NEFF leaves device state (semaphores) behind and corrupts the second run,
    so each kernel() invocation gets a fresh executable (BIR->NEFF is
    disk-cached, so this costs seconds, not a recompile)."""
    from concourse import bass2jax
    from concourse.bass2jax import _bass_exec_p, partition_id_tensor

    if "nc" in _COMPILED:
        nc = _COMPILED["nc"]
    else:
        nc = build_nc()
        _COMPILED["nc"] = nc
    bass2jax.install_neuronx_cc_hook()
    partition_name = (nc.partition_id_tensor.name
                      if nc.partition_id_tensor else None)
    in_names, out_names, out_avals, zero_outs = [], [], [], []
    for alloc in nc.m.functions[0].allocations:
        if not isinstance(alloc, mybir.MemoryLocationSet):
            continue
        nm = alloc.memorylocations[0].name
        if alloc.kind == "ExternalInput":
            if nm != partition_name:
                in_names.append(nm)
        elif alloc.kind == "ExternalOutput":
            out_names.append(nm)
            out_avals.append(_jax.core.ShapedArray(
                tuple(alloc.tensor_shape), mybir.dt.np(alloc.dtype)))
            zero_outs.append(np.zeros(tuple(alloc.tensor_shape),
                                      mybir.dt.np(alloc.dtype)))
    n_params = len(in_names)
    n_outs = len(out_avals)
    in_names_all = in_names + out_names
    if partition_name is not None:
        in_names_all.append(partition_name)
    donate = tuple(range(n_params, n_params + n_outs))

    def _body(*args):
        operands = list(args)
        if partition_name is not None:
            operands.append(partition_id_tensor())
        return tuple(_bass_exec_p.bind(
            *operands, out_avals=tuple(out_avals),
            in_names=tuple(in_names_all), out_names=tuple(out_names),
            lowering_input_output_aliases=(), sim_require_finite=True,
            sim_require_nnan=True, nc=nc))

    devices = _jax.devices()[:8]
    mesh = _Mesh(np.asarray(devices), ("core",))
    in_specs = (_PartitionSpec("core"),) * (n_params + n_outs)
    out_specs = (_PartitionSpec("core"),) * len(out_names)
    sharded = _jax.jit(
        _shard_map(_body, mesh=mesh, in_specs=in_specs, out_specs=out_specs,
                   check_rep=False),
        donate_argnums=donate, keep_unused=True)
    return (sharded, in_names, out_names, zero_outs)


def kernel(**inputs):
    """Full-input GNN forward on 8 TRN2 NeuronCores; returns [64, 4] f32."""
    sharded, in_names, out_names, zero_outs = _build_and_jit()
    core_maps = prep_inputs(inputs)
    concat_in = [np.concatenate([core_maps[c][nm] for c in range(8)], axis=0)
                 for nm in in_names]
    concat_zero = [np.zeros((8 * z.shape[0], *z.shape[1:]), z.dtype)
                   for z in zero_outs]
    out_arrs = sharded(*concat_in, *concat_zero)
    oi = out_names.index("out")
    full = np.asarray(out_arrs[oi]).reshape(8, 4, G)
    return np.concatenate([full[c].T for c in range(8)], axis=0)


# revision 101
# speedup vs baseline: 1.0455x; 1.0137x over previous
"""GNN (GENConv x2 + TopK pool) Bass/Tile kernel for TRN2, data-parallel over
8 NeuronCores (8 graphs per core).

Edge aggregation uses a fixed-degree main layout: node n owns edge slots
[32n, 32n+32) (ghost-padded, corrected analytically via host-known pad
counts), so the scatter-softmax segment sums become strided windowed
tensor_reduce ops -- no prefix-scan carry chains, no big boundary gathers.
Overflow edges (deg > 32, ~7%) go to a 2048-slot dst-sorted spill region per
graph handled by a small scan + boundary gather.

Per-core layout ("fm" = feature-major packed): edge tensors are
[128 = 16feat x 8graph, 18432 slots]; node tensors [128, 512].
"""

import numpy as np
from contextlib import ExitStack

import concourse.bass as bass
import concourse.bacc as bacc
import concourse.mybir as mybir
import concourse.tile as tile
from concourse import library_config

F32 = mybir.dt.float32
F32R = mybir.dt.float32r
F16 = mybir.dt.float16
F8E4 = mybir.dt.float8e4
I16 = mybir.dt.int16
I8 = mybir.dt.int8

G = 8          # graphs per core
N = 512        # nodes per graph
DEG = 32       # fixed main-slot degree per node
EG = 16384     # edges per graph (input)
EM = N * DEG   # main slots per graph = 16384
ESP = 1536     # spill slots per graph (max observed 1248)
ET = EM + ESP  # total slots = 18432
EF = 16        # edge/node feature dim after encode
XF = 64        # input node feature dim
K = 256        # topk keep
CHUNK = 1024   # slots per chunk
NCH = ET // CHUNK        # 18 chunks (16 main + 2 spill)
NMAIN = EM // CHUNK      # 16
NT = 528       # gather table columns (512 nodes + ghost pad; 16-multiple)
GHOST = 512
NEND = 576     # padded spill end-list length (513 used)
EPS = 1e-7
BIGNEG = 6.0e4
LN32 = float(np.log(DEG))

import os as _os
STOP_STAGE = _os.environ.get("K_STOP_STAGE") or None  # hw bisect hook


# ----------------------------------------------------------------------------
# Host-side preprocessing: full inputs -> per-core named arrays
# ----------------------------------------------------------------------------

def prep_inputs(inputs: dict) -> list[dict]:
    x = np.asarray(inputs["x"], np.float32)            # [B*N, 64]
    ei = np.asarray(inputs["edge_index"])              # [2, E] int64
    ea = np.asarray(inputs["edge_attr"], np.float32)   # [E, 16]
    B = 64
    assert x.shape == (B * N, XF)
    assert ea.shape == (B * EG, EF)

    src_g = (ei[0] % N).astype(np.int64)
    dst_g = (ei[1] % N).astype(np.int64)
    graph_of_edge = (ei[0] // N).astype(np.int64)
    assert np.array_equal(graph_of_edge, np.repeat(np.arange(B), EG)), \
        "edge blocks not per-graph; prep assumes reference setup_inputs layout"
    assert np.array_equal(ei[0] // N, ei[1] // N)

    def lin(name):
        return np.asarray(inputs[name], np.float32)

    W_ne, b_ne = lin("W_ne"), lin("b_ne")
    W_ee, b_ee = lin("W_ee"), lin("b_ee")
    W1a, b1a, g1, be1 = lin("W1a"), lin("b1a"), lin("g1"), lin("be1")
    W1b, b1b = lin("W1b"), lin("b1b")
    W2a, b2a, g2, be2 = lin("W2a"), lin("b2a"), lin("g2"), lin("be2")
    W2b, b2b = lin("W2b"), lin("b2b")
    Wa, ba, Wo, bo = lin("Wa"), lin("ba"), lin("Wo"), lin("bo")
    w_pool = lin("w_pool")
    wp = w_pool / np.linalg.norm(w_pool)
    t1 = np.float32(inputs["t1"])
    t2 = np.float32(inputs["t2"])

    # centering fold: LN(y) uses yC = y - mean(y) = u @ (W C) + b C
    C32 = np.eye(32, dtype=np.float32) - 1.0 / 32.0
    W1aC = W1a @ C32
    b1aC = b1a @ C32
    W2aC = W2a @ C32
    b2aC = b2a @ C32

    cst = {}
    wne = np.zeros((128, 4 * 128), np.float32)
    for p in range(4):
        for a in range(2):
            gg = 2 * p + a
            wne[64 * a:64 * a + XF, 128 * p + 16 * gg:128 * p + 16 * gg + EF] = W_ne
    cst["wne_stat"] = wne.astype(np.float16)
    cst["bne_vec"] = np.tile(b_ne, G)[:, None].astype(np.float32)
    wee = np.zeros((128, 128), np.float32)
    for g in range(G):
        wee[16 * g:16 * g + EF, 16 * g:16 * g + EF] = W_ee
    cst["wee_stat"] = wee.astype(np.float16)
    cst["bee_vec"] = np.tile(b_ee, G)[:, None].astype(np.float32)
    cst["mln32_vec"] = np.full((128, 1), -LN32, np.float32)
    cst["identT"] = np.eye(128, dtype=np.float32)
    cst["ident"] = np.eye(128, dtype=np.float32)
    cst["t1vec"] = np.full((128, 1), t1, np.float32)
    cst["t2vec"] = np.full((128, 1), t2, np.float32)
    w1a = np.zeros((64, 128), np.float32)
    for gg in range(4):
        w1a[16 * gg:16 * gg + 16, 32 * gg:32 * gg + 32] = W1aC
    cst["w1a_stat"] = np.vstack([w1a, w1a])
    cst["b1a_vec"] = np.tile(b1aC, 4)[:, None].astype(np.float32)
    ones32 = np.zeros((128, 4), np.float32)
    for gg in range(4):
        ones32[32 * gg:32 * gg + 32, gg] = 1.0 / 32.0
    cst["ones32h_stat"] = ones32.astype(np.float16)
    onesb32 = np.zeros((4, 128), np.float32)
    for gg in range(4):
        onesb32[gg, 32 * gg:32 * gg + 32] = 1.0
    cst["onesb32_stat"] = onesb32
    cst["g1_vec"] = np.tile(g1, 4)[:, None].astype(np.float32)
    cst["be1_vec"] = np.tile(be1, 4)[:, None].astype(np.float32)
    w1b = np.zeros((64, 32), np.float32)
    for gg in range(2):
        w1b[32 * gg:32 * gg + 32, 16 * gg:16 * gg + 16] = W1b
    cst["w1bh_stat"] = np.vstack([w1b, w1b]).astype(np.float16)
    cst["b1b_vec"] = np.tile(b1b, G)[:, None].astype(np.float32)
    wpool = np.zeros((128, 8), np.float32)
    for g in range(G):
        wpool[16 * g:16 * g + EF, g] = wp
    cst["wpool_stat"] = wpool
    ones16b = np.zeros((8, 128), np.float32)
    for g in range(G):
        ones16b[g, 16 * g:16 * g + EF] = 1.0
    cst["ones16b_stat"] = ones16b.astype(np.float16)
    o16s = np.zeros((40, 128), np.float32)
    o16s[32:40] = ones16b
    cst["ones16b32_stat"] = o16s.astype(np.float16)
    onesel = np.zeros((8, 8 * 128), np.float32)
    for g in range(8):
        onesel[g, 128 * g:128 * (g + 1)] = 1.0
    cst["onesel_stat"] = onesel
    w2a = np.zeros((64, 128), np.float32)
    for gg in range(4):
        w2a[16 * gg:16 * gg + 16, 32 * gg:32 * gg + 32] = W2aC
    cst["w2a_stat"] = np.vstack([w2a, w2a])
    cst["b2a_vec"] = np.tile(b2aC, 4)[:, None].astype(np.float32)
    cst["g2_vec"] = np.tile(g2, 4)[:, None].astype(np.float32)
    cst["be2_vec"] = np.tile(be2, 4)[:, None].astype(np.float32)
    w2b = np.zeros((64, 64), np.float32)
    for gg in range(2):
        w2b[32 * gg:32 * gg + 32, 32 * gg:32 * gg + 32] = W2b
    cst["w2bh_stat"] = np.vstack([w2b, w2b]).astype(np.float16)
    cst["b2b_vec"] = np.tile(b2b, 4)[:, None].astype(np.float32)
    mbA = np.zeros((8, 128), np.float32)
    mbB = np.zeros((8, 128), np.float32)
    for g in range(4):
        mbA[g, 32 * g:32 * g + 32] = 1.0
        mbB[g + 4, 32 * g:32 * g + 32] = 1.0
    mbA64 = np.zeros((72, 128), np.float32); mbA64[64:72] = mbA
    mbB64 = np.zeros((72, 128), np.float32); mbB64[64:72] = mbB
    cst["maskbc_statA"] = mbA64.astype(np.float16)
    cst["maskbc_statB"] = mbB64.astype(np.float16)
    selk = np.zeros((128, 4 * 32), np.float32)
    for gg in range(4):
        selk[32 * gg:32 * gg + 32, 32 * gg:32 * gg + 32] = np.eye(32) / K
    cst["selk_stat"] = selk
    d8 = np.zeros((8, 64), np.float32)
    for g in range(G):
        d8[g, 9 * g] = 1.0   # spreads transpose(mask) onto per-graph columns
    cst["d8_stat"] = d8
    cst["wa_stat"] = Wa.astype(np.float32)
    cst["ba_vec"] = ba[:, None].astype(np.float32)
    cst["wo_stat"] = Wo.astype(np.float32)
    cst["bo2_vec"] = (-2.0 * bo)[:, None].astype(np.float32)
    cst["lneps_vec"] = np.full((4, 1), 1e-5, np.float32)

    core_maps = []
    for core in range(8):
        m = dict(cst)
        gsl = slice(core * G, (core + 1) * G)
        xt = np.zeros((128, 4 * 512), np.float32)
        xs = x.reshape(B, N, XF)[gsl]
        for p in range(4):
            for a in range(2):
                xt[64 * a:64 * a + XF, 512 * p:512 * (p + 1)] = xs[2 * p + a].T
        m["xT"] = xt.astype(np.float16)

        attrT = np.zeros((128, ET), np.float16)
        srcidx = np.zeros((128, ET // 16), np.int16)
        spendidx = np.zeros((128, NEND // 16), np.int16)
        padcnt = np.zeros((128, N), np.float16)
        amat = np.zeros((128, 32 * 512), np.float16)
        for gl in range(G):
            gid = core * G + gl
            s_l = src_g[gid * EG:(gid + 1) * EG]
            d_l = dst_g[gid * EG:(gid + 1) * EG]
            order = np.argsort(d_l, kind="stable")
            ds = d_l[order]
            ss = s_l[order]
            ats = ea[gid * EG:(gid + 1) * EG][order]     # [EG, 16] dst-sorted
            counts = np.bincount(ds, minlength=N)
            starts = np.zeros(N + 1, np.int64)
            np.cumsum(counts, out=starts[1:])
            # main: node n gets its first min(deg,32) edges at slots 32n+o
            j = np.arange(EM)
            nn = j // DEG
            oo = j % DEG
            msk = oo < np.minimum(counts[nn], DEG)
            pos = starts[nn] + oo
            srcf = np.full(ET, GHOST, np.int64)
            attrf = np.zeros((ET, EF), np.float32)
            srcf[j[msk]] = ss[pos[msk]]
            attrf[j[msk]] = ats[pos[msk]]
            # spill: rank within dst-run >= 32 (already dst-sorted)
            r = np.arange(EG) - starts[ds]
            spm = r >= DEG
            nsp = int(spm.sum())
            assert nsp <= ESP, f"spill overflow: {nsp} > {ESP}"
            srcf[EM:EM + nsp] = ss[spm]
            attrf[EM:EM + nsp] = ats[spm]
            spd = ds[spm]
            e_sp = np.searchsorted(spd, np.arange(N), side="right")
            elist = np.zeros(NEND, np.int16)
            elist[1:N + 1] = e_sp.astype(np.int16)

            attrT[16 * gl:16 * gl + EF, :] = attrf.T.astype(np.float16)
            srcidx[16 * gl:16 * gl + 16, :] = \
                srcf.astype(np.int16).reshape(ET // 16, 16).T
            spendidx[16 * gl:16 * gl + 16, :] = \
                elist.reshape(NEND // 16, 16).T
            padcnt[16 * gl:16 * gl + 16, :] = \
                np.maximum(0, DEG - counts)[None, :].astype(np.float16)
            # adjacency count matrix A[m, n] = #edges m->n (for the L2
            # masked-src denominator correction via matmul)
            A = np.zeros((N, N), np.int32)
            np.add.at(A, (s_l, d_l), 1)
            assert A.max() <= 2048, A.max()  # f16 integers exact to 2048
            for b in range(4):
                amat[:, (4 * gl + b) * 512:(4 * gl + b + 1) * 512] = \
                    A[128 * b:128 * (b + 1), :].astype(np.float16)
        m["spendidx"] = spendidx
        m["padcnt"] = padcnt
        blob = np.zeros((128, CBLOB_BYTES), np.uint8)
        for name, shape, dt, off in CONST_SPECS:
            arr = m[name]
            bv = arr.view(np.uint8).reshape(arr.shape[0], -1)
            blob[:arr.shape[0], off:off + bv.shape[1]] = bv
        rblob = np.zeros((128, 4 * 128), np.float32)
        rblob[:, 0:128] = m["ident"]
        rblob[:, 128:256] = m["w1a_stat"]
        rblob[:, 256:384] = m["w2a_stat"]
        rblob[0:4, 384:512] = m["onesb32_stat"]
        core_maps.append({"cblob": blob, "rblob": rblob, "attrT": attrT,
                          "xT": m["xT"], "srcidx": srcidx, "amat": amat})
    return core_maps


_CSPEC_RAW = [
    # encode-critical constants first (covered by the first cblob DMA piece)
    ("wne_stat", [128, 4 * 128], F16),
    ("bne_vec", [128, 1], F32),
    ("wee_stat", [128, 128], F16),
    ("bee_vec", [128, 1], F32),
    ("mln32_vec", [128, 1], F32),
    ("t1vec", [128, 1], F32),
    ("t2vec", [128, 1], F32),
    ("spendidx", [128, NEND // 16], I16),
    ("padcnt", [128, N], F16),
    # ---- split point: everything below arrives with the second DMA ----
    ("identT", [128, 128], F32),
    ("b1a_vec", [128, 1], F32),
    ("ones32h_stat", [128, 4], F16),
    ("g1_vec", [128, 1], F32),
    ("be1_vec", [128, 1], F32),
    ("w1bh_stat", [128, 32], F16),
    ("b1b_vec", [128, 1], F32),
    ("wpool_stat", [128, 8], F32),
    ("ones16b_stat", [8, 128], F16),
    ("ones16b32_stat", [40, 128], F16),
    ("onesel_stat", [8, 8 * 128], F32),
    ("b2a_vec", [128, 1], F32),
    ("g2_vec", [128, 1], F32),
    ("be2_vec", [128, 1], F32),
    ("w2bh_stat", [128, 64], F16),
    ("b2b_vec", [128, 1], F32),
    ("maskbc_statA", [72, 128], F16),
    ("maskbc_statB", [72, 128], F16),
    ("selk_stat", [128, 4 * 32], F32),
    ("d8_stat", [8, 64], F32),
    ("wa_stat", [32, 128], F32),
    ("ba_vec", [128, 1], F32),
    ("wo_stat", [128, 4], F32),
    ("bo2_vec", [4, 1], F32),
    ("lneps_vec", [4, 1], F32),
]

_SPLIT_AFTER = "padcnt"


def _mk_const_specs():
    specs = []
    off = 0
    for nm, shape, dt in _CSPEC_RAW:
        nbytes = int(np.prod(shape[1:])) * mybir.dt.size(dt)
        specs.append((nm, shape, dt, off))
        off += (nbytes + 127) // 128 * 128
    return specs, off

CONST_SPECS, CBLOB_BYTES = _mk_const_specs()

INPUT_SPECS = [
    ("cblob", [128, CBLOB_BYTES], mybir.dt.uint8),
    ("rblob", [128, 4 * 128], F32R),   # ident | w1a | w2a | onesb32(rows 0:4)
    ("attrT", [128, ET], F16),
    ("xT", [128, 4 * 512], F16),
    ("srcidx", [128, ET // 16], I16),
    ("amat", [128, 32 * 512], F16),
]


# ----------------------------------------------------------------------------
# Device graph
# ----------------------------------------------------------------------------

def build_nc(debug_keys=(), n_rep=1):
    nc = bacc.Bacc(None, target_bir_lowering=False, debug=False)
    A = {}
    for name, shape, dt in INPUT_SPECS:
        A[name] = nc.declare_dram_parameter(name, shape, dt, isOutput=False)[:]
    out_ext = nc.declare_dram_parameter("out", [4, G], F32, isOutput=True)[:]
    dbg_ext = {}
    dbg_shapes = {
        "hT": ((128, N), F32), "mc0": ((128, CHUNK), F16),
        "ev0": ((128, CHUNK), F16), "evm0": ((128, CHUNK), F16),
        "sums0": ((128, N), F16), "sums1": ((128, N), F16),
        "spev0": ((128, ESP), F16), "Ssp0": ((128, ESP + 1), F32),
        "Dsp0": ((128, N), F32), "Dsp1": ((128, N), F32),
        "aggr1": ((128, N), F32), "u1": ((128, N), F32),
        "h1": ((128, N), F32),
        "score": ((8, N), F32), "rk": ((128, 32), F32),
        "mask": ((128, 32), F32),
        "hq": ((128, N), F32), "hp": ((128, N), F32),
        "aggr2": ((128, N), F32), "h2a": ((128, N), F16),
        "h2b": ((128, N), F16), "pooled": ((32, 8), F32),
        "y1sA": ((128, N), F16), "r1A": ((128, N), F16),
    }
    for key in debug_keys:
        shape, dt = dbg_shapes[key]
        dbg_ext[key] = nc.declare_dram_parameter(
            "dbg_" + key, list(shape), dt, isOutput=True)[:]

    with tile.TileContext(nc) as tc, ExitStack() as ctx:
        consts = ctx.enter_context(tc.tile_pool(name="consts", bufs=1))
        nodep = ctx.enter_context(tc.tile_pool(name="nodep", bufs=1))
        work = ctx.enter_context(tc.tile_pool(name="work", bufs=2))
        psum = ctx.enter_context(tc.tile_pool(name="psum", bufs=2, space="PSUM"))
        psum1 = ctx.enter_context(tc.tile_pool(name="psum1", bufs=1, space="PSUM"))
        psume = ctx.enter_context(tc.tile_pool(name="psume", bufs=1, space="PSUM"))
        psumw = ctx.enter_context(tc.tile_pool(name="psumw", bufs=1, space="PSUM"))

        nc.gpsimd.load_library(library_config.ap_gather)

        # ---- DMAs: xT first (encode-critical), then cblob, srcidx, attr ----
        xts = consts.tile([128, 4 * 512], F16, name="xT_sb")
        nc.sync.dma_start(out=xts, in_=A["xT"])
        cb = consts.tile([128, CBLOB_BYTES], mybir.dt.uint8, name="cblob_sb")
        _split = next(off for nm, _s, _d, off in CONST_SPECS
                      if nm == "identT")
        nc.sync.dma_start(out=cb[:, :_split], in_=A["cblob"][:, :_split])
        srcidx_sb = consts.tile([128, ET // 16], I16, name="srcidx_sb")
        nc.sync.dma_start(out=srcidx_sb, in_=A["srcidx"])
        attr_all = consts.tile([128, ET], F16, name="attr_sb")
        # pieces in chunk-consumption order: main[0:2048], spill, rest
        nc.sync.dma_start(out=attr_all[:, 0:2048], in_=A["attrT"][:, 0:2048])
        rb_sb = consts.tile([128, 4 * 128], F32R, name="rblob_sb")
        nc.sync.dma_start(out=rb_sb, in_=A["rblob"])
        nc.sync.dma_start(out=attr_all[:, EM:], in_=A["attrT"][:, EM:])
        for lo, hi in ((2048, 8192), (8192, 12288), (12288, EM)):
            nc.sync.dma_start(out=attr_all[:, lo:hi], in_=A["attrT"][:, lo:hi])
        nc.sync.dma_start(out=cb[:, _split:], in_=A["cblob"][:, _split:])
        amat_sb = consts.tile([128, 16 * 512], F16, name="amat_sb")
        nc.sync.dma_start(out=amat_sb, in_=A["amat"][:, :16 * 512])

        C = {"srcidx": srcidx_sb, "attr": attr_all, "amat": amat_sb,
             "amat_dram": A["amat"],
             "ident": rb_sb[:, 0:128], "w1a_stat": rb_sb[:, 128:256],
             "w2a_stat": rb_sb[:, 256:384],
             "onesb32_stat": rb_sb[0:4, 384:512]}
        for name, shape, dt, off in CONST_SPECS:
            nbytes = int(np.prod(shape[1:])) * mybir.dt.size(dt)
            ap = cb[:shape[0], off:off + nbytes].bitcast(dt)
            if len(shape) == 3:
                ap = ap.rearrange("p (a b) -> p a b", b=shape[2])
            C[name] = ap

        dbg_keys_set = set(debug_keys)

        def dbg(key, ap):
            if key in dbg_keys_set:
                nc.sync.dma_start(out=dbg_ext[key], in_=ap)

        for rep in range(n_rep):
            run_once(nc, tc, A, C, xts, out_ext, dbg, nodep, work,
                     psum, psum1, psume, psumw, rep)
    nc.compile()
    return nc


def run_once(nc, tc, A, C, xts, out_ext, dbg, nodep, work, psum, psum1,
             psume, psumw, rep):
    V = nc.vector
    S = nc.scalar
    T = nc.tensor
    Alu = mybir.AluOpType
    Act = mybir.ActivationFunctionType

    # ================= node encode: hT = x @ W_ne + b_ne =================
    hraw = psume.tile([128, N], F32, name="hraw", tag="e512")
    for p in range(4):
        T.matmul(hraw, C["wne_stat"][:, 128 * p:128 * (p + 1)],
                 xts[:, 512 * p:512 * (p + 1)],
                 start=(p == 0), stop=(p == 3))
    hT = nodep.tile([128, NT, 1], F32, name=f"hT_{rep}")
    S.activation(hT[:, :N, 0], hraw, Act.Identity, bias=C["bne_vec"], scale=1.0)
    S.activation(hT[:, N:, 0], hraw[:, 0:NT - N], Act.Copy, bias=0.0, scale=0.0)
    dbg("hT", hT[:, :N, 0])

    def stop_dma(ap):
        nc.sync.dma_start(out=out_ext, in_=ap)

    if STOP_STAGE == "enc":
        return stop_dma(hT[0:4, 0:G, 0].bitcast(F32))

    # ================= shared edge-phase machinery =================
    def edge_layer(layer, table, tvec, bee, mm=None, dbg_on=False,
                   inject=None):
        """Returns aggr tile [128, N] f32 (softmax-weighted mean + pad fix)."""
        sums = nodep.tile([128, 2, N], F32, name=f"sums_{layer}_{rep}",
                          tag="sums")
        spev = nodep.tile([128, 2, ESP], F16, name=f"spev_{layer}_{rep}",
                          tag="spev")
        # ev for ghost-pad slots: mc_pad = relu(bee); ev_pad = exp(t*mc - ln32)
        beer = nodep.tile([128, 1], F32, name=f"beer_{layer}_{rep}", tag="beer")
        S.activation(beer, bee, Act.Relu, bias=0.0, scale=1.0)
        evpad = nodep.tile([128, 1], F32, name=f"evpad_{layer}_{rep}",
                           tag="evpad")
        S.activation(evpad, beer, Act.Exp, bias=C["mln32_vec"], scale=tvec)
        evmpad = nodep.tile([128, 1], F32, name=f"evmpad_{layer}_{rep}",
                            tag="evmpad")
        V.tensor_tensor(out=evmpad, in0=evpad, in1=beer, op=Alu.mult)

        # 2 main chunks prime the pipeline, then spill (so its scan +
        # boundary-gather tail overlaps the remaining main chunks)
        E = (sums, None, evpad, evmpad)
        for cc in [0, 1, NMAIN, NMAIN + 1] + list(range(2, NMAIN)):
            if inject is not None and cc == inject[0]:
                E = (sums, Dsp, evpad, evmpad)
                inject[1](E)
            base = cc * CHUNK
            csz = CHUNK if cc != NMAIN + 1 else ESP - CHUNK
            hsrc = work.tile([128, CHUNK, 1], F32, name="hsrc", tag="hsrc")
            j0 = base // 16
            nc.gpsimd.ap_gather(
                hsrc[:, :csz, :], table, C["srcidx"][:, j0:j0 + csz // 16],
                channels=128, num_elems=NT, d=1, num_idxs=csz)
            if STOP_STAGE == "pipe1a" and layer == 0:
                return None
            zc = psum.tile([128, CHUNK], F32, name="zc", tag="zc")
            for s in range(csz // 512):
                sl = slice(512 * s, 512 * (s + 1))
                T.matmul(zc[:, sl], C["wee_stat"],
                         C["attr"][:, base + 512 * s:base + 512 * (s + 1)],
                         start=True, stop=False, skip_group_check=True)
                T.matmul(zc[:, sl], C["ident"].bitcast(F32), hsrc[:, sl, 0],
                         start=False, stop=True, skip_group_check=True)
            mc = work.tile([128, CHUNK], F16, name="mc", tag="mc")
            S.activation(mc[:, :csz], zc[:, :csz], Act.Relu, bias=bee,
                         scale=1.0)
            if STOP_STAGE == "pipe1b" and layer == 0:
                return None
            if cc < NMAIN:
                evcc = work.tile([128, 2, CHUNK], F16, name="evcc", tag="evcc")
                ev_t, evm_t = evcc[:, 0, :], evcc[:, 1, :]
            else:
                sp = cc - NMAIN
                spsl = slice(CHUNK * sp, CHUNK * sp + csz)
                ev_t, evm_t = spev[:, 0, spsl], spev[:, 1, spsl]
            mc = mc[:, :csz]
            # L2 masked-src edges (hq[src] = -2e9): mc = 0 exactly, so they
            # add 0 to the numerator and exactly 1/32 to the denominator;
            # the denominator excess is removed analytically via `mm`
            # (adjacency matmul) instead of a per-edge mask multiply.
            S.activation(ev_t, mc, Act.Exp, bias=C["mln32_vec"], scale=tvec)
            V.tensor_tensor(out=evm_t, in0=ev_t, in1=mc, op=Alu.mult)
            if STOP_STAGE == "pipe1" and layer == 0:
                return None
            if cc < NMAIN:
                # windowed segment-sum via pairwise folds (f16 2x DVE mode)
                # + small f32 reduce: ~1550ns vs 2254 for one big 1x reduce
                f1 = work.tile([128, 2, 32, 16], F16, name="f1", tag="f1")
                ev4 = evcc[:, :, :].rearrange("p a (b c) -> p a b c", c=DEG)
                V.tensor_tensor(out=f1, in0=ev4[:, :, :, 0:16],
                                in1=ev4[:, :, :, 16:32], op=Alu.add)
                f2 = work.tile([128, 2, 32, 8], F16, name="f2", tag="f2")
                V.tensor_tensor(out=f2, in0=f1[:, :, :, 0:8],
                                in1=f1[:, :, :, 8:16], op=Alu.add)
                f3 = work.tile([128, 2, 32, 4], F16, name="f3", tag="f3")
                V.tensor_tensor(out=f3, in0=f2[:, :, :, 0:4],
                                in1=f2[:, :, :, 4:8], op=Alu.add)
                V.tensor_reduce(
                    out=sums[:, :, DEG * cc:DEG * (cc + 1)],
                    in_=f3, axis=mybir.AxisListType.X, op=Alu.add)
            if dbg_on and cc == 0:
                dbg("mc0", mc)
                dbg("ev0", ev_t)
                dbg("evm0", evm_t)
            if cc == NMAIN + 1:
                # spill scans + boundary gather (overlap with main chunks)
                Dsp = []
                for ti in range(2):
                    Ssp = nodep.tile([128, ESP + 1, 1], F32,
                                     name=f"Ssp{ti}_{layer}_{rep}",
                                     tag=f"Ssp{ti}")
                    V.memset(Ssp[:, 0:1, 0], 0.0)
                    V.tensor_tensor_scan(
                        out=Ssp[:, 1:, 0], data0=spev[:, ti, :],
                        data1=spev[:, ti, :], initial=0.0,
                        op0=Alu.add, op1=Alu.bypass)
                    gsp = nodep.tile([128, NEND, 1], F32,
                                     name=f"gsp{ti}_{layer}_{rep}",
                                     tag=f"gsp{ti}")
                    nc.gpsimd.ap_gather(
                        gsp, Ssp, C["spendidx"],
                        channels=128, num_elems=ESP + 1, d=1, num_idxs=NEND)
                    d = nodep.tile([128, N], F32, name=f"Dsp{ti}_{layer}_{rep}",
                                   tag=f"Dsp{ti}")
                    V.tensor_tensor(out=d, in0=gsp[:, 1:N + 1, 0],
                                    in1=gsp[:, 0:N, 0], op=Alu.subtract)
                    Dsp.append(d)
                if dbg_on:
                    dbg("spev0", spev[:, 0, :])
                    dbg("Ssp0", Ssp[:, :, 0])
                    dbg("Dsp0", Dsp[0])
                    dbg("Dsp1", Dsp[1])

        if dbg_on:
            dbg("sums0", sums[:, 0, :])
            dbg("sums1", sums[:, 1, :])
        return (sums, Dsp, evpad, evmpad)

    def aggr_half(E, hs, base_in, mm, layer, hi):
        """Softmax-mean for node slice hs; returns u = base + aggr + EPS."""
        sums, Dsp, evpad, evmpad = E
        HN = hs.stop - hs.start
        neg = []
        for ti, padv in ((0, evpad), (1, evmpad)):
            tot = nodep.tile([128, HN], F32, name=f"tot{ti}_{layer}{hi}_{rep}",
                             tag=f"tot{ti}")
            V.tensor_tensor(out=tot, in0=sums[:, ti, hs], in1=Dsp[ti][:, hs],
                            op=Alu.add)
            if ti == 0 and mm is not None:
                # tot += (mask-1)^T A / 32 (mm pre-scaled by 1/32)
                mmb = psume.tile([128, HN], F32, name="mmb", tag="e512")
                T.matmul(mmb, C["ones16b_stat"], mm[:, hs],
                         start=True, stop=True)
                tot2 = nodep.tile([128, HN], F32,
                                  name=f"tot2_{layer}{hi}_{rep}", tag="tot2")
                V.tensor_tensor(out=tot2, in0=mmb, in1=tot, op=Alu.add)
                tot = tot2
            ng = nodep.tile([128, HN], F32, name=f"ng{ti}_{layer}{hi}_{rep}",
                            tag=f"ng{ti}")
            V.scalar_tensor_tensor(out=ng, in0=C["padcnt"][:, hs], scalar=padv,
                                   in1=tot, op0=Alu.mult, op1=Alu.subtract)
            neg.append(ng)
        dm = nodep.tile([128, HN], F32, name=f"dm_{layer}{hi}_{rep}", tag="dm")
        V.tensor_scalar(dm, neg[0], -1e-16, None, Alu.min)
        rec = nodep.tile([128, HN], F32, name=f"rec_{layer}{hi}_{rep}",
                         tag="rec")
        V.reciprocal(rec, dm)
        ag = nodep.tile([128, HN], F32, name=f"ag_{layer}{hi}_{rep}", tag="ag")
        V.tensor_tensor(out=ag, in0=neg[1], in1=rec, op=Alu.mult)
        u = nodep.tile([128, HN], F32R, name=f"u_{layer}{hi}_{rep}", tag="u")
        V.scalar_tensor_tensor(out=u, in0=ag, scalar=EPS, in1=base_in[:, hs],
                               op0=Alu.add, op1=Alu.add)
        return u

    def mlp_half(uin, hs, wa_stat, ba_vec, gvec, bevec, wb_stat, layer, hi):
        HN = hs.stop - hs.start
        N2 = 2 * HN
        y1p = psumw.tile([128, 2, HN], F32, name=f"y1p_{layer}{hi}",
                         tag="wide")
        for half in range(2):
            T.matmul(y1p[:, half, :],
                     wa_stat[64 * half:64 * half + 64, :],
                     uin[64 * half:64 * half + 64, :],
                     start=True, stop=True)
        y1pf = y1p.rearrange("p a b -> p (a b)")
        y1s = nodep.tile([128, N2], F16, name=f"y1s_{layer}{hi}_{rep}",
                         tag="y1s")
        S.activation(y1s, y1pf, Act.Identity, bias=ba_vec, scale=1.0)
        sq = nodep.tile([128, N2], F16, name=f"sq_{layer}{hi}_{rep}", tag="sq")
        V.tensor_tensor(out=sq, in0=y1s, in1=y1s, op=Alu.mult)
        vp = psumw.tile([4, 2, HN], F32, name=f"vp_{layer}{hi}", tag="wide")
        for half in range(2):
            T.matmul(vp[:, half, :], C["ones32h_stat"],
                     sq[:, half * HN:half * HN + HN], start=True, stop=True)
        lnv = nodep.tile([4, N2], F32, name=f"lnv_{layer}{hi}_{rep}",
                         tag="st4", bufs=2)
        S.activation(lnv, vp.rearrange("p a b -> p (a b)"), Act.Ln,
                     bias=C["lneps_vec"], scale=1.0)
        rstd = nodep.tile([4, N2], F32R, name=f"rstd_{layer}{hi}_{rep}",
                          tag="st4", bufs=2)
        S.activation(rstd, lnv, Act.Exp, bias=0.0, scale=-0.5)
        rb = psumw.tile([128, 2, HN], F32, name=f"rb_{layer}{hi}", tag="wide")
        for half in range(2):
            T.matmul(rb[:, half, :], C["onesb32_stat"],
                     rstd[:, half * HN:half * HN + HN],
                     start=True, stop=True)
        vnorm = nodep.tile([128, N2], F32, name=f"vn_{layer}{hi}_{rep}",
                           tag="vn")
        V.tensor_tensor(out=vnorm, in0=y1s,
                        in1=rb.rearrange("p a b -> p (a b)"), op=Alu.mult)
        r1 = nodep.tile([128, N2], F16, name=f"r1_{layer}{hi}_{rep}", tag="r1")
        S.activation(r1, vnorm, Act.Relu, bias=bevec, scale=gvec)
        M = wb_stat.shape[1]
        outs = []
        for half in range(2):
            for q in range(2):
                yq = psum.tile([M, HN], F32, name=f"yq{half}{q}", tag="zc")
                T.matmul(yq, wb_stat[64 * q:64 * q + 64, :],
                         r1[64 * q:64 * q + 64, half * HN:half * HN + HN],
                         start=True, stop=True)
                outs.append(yq)
        return outs

    HALVES = (slice(0, N),)

    # ================= Layer 1 =================
    h1 = nodep.tile([128, N], F32, name=f"h1_{rep}", tag="h1")

    def half_pipe(layer, base_in, mm, wa, ba, gv, bev, wb, hout):
        def emit(E, hi, hs):
            u = aggr_half(E, hs, base_in, mm, layer, hi)
            y2q = mlp_half(u, hs, wa, ba, gv, bev, wb, layer, hi)
            for q in range(4):
                if layer == 0:
                    S.activation(hout[32 * q:32 * q + 32, hs], y2q[q],
                                 Act.Relu,
                                 bias=C["b1b_vec"][32 * q:32 * q + 32, :],
                                 scale=1.0)
                else:
                    sl, part = q // 2, q % 2
                    S.activation(hout[sl][64 * part:64 * part + 64, hs],
                                 y2q[q], Act.Relu,
                                 bias=C["b2b_vec"][64 * part:64 * part + 64, :],
                                 scale=1.0)
        return emit

    emit1 = half_pipe(0, hT[:, :N, 0], None, C["w1a_stat"], C["b1a_vec"],
                      C["g1_vec"], C["be1_vec"], C["w1bh_stat"], h1)
    E1 = edge_layer(0, hT, C["t1vec"], C["bee_vec"], dbg_on=True)
    if STOP_STAGE in ("pipe1", "pipe1a", "pipe1b"):
        return None
    emit1(E1, 0, HALVES[0])
    dbg("h1", h1)
    if STOP_STAGE == "mlp1":
        return stop_dma(h1[0:4, 0:G])

    # ================= score / topk mask / gates =================
    scp = psume.tile([8, N], F32, name="scp", tag="e512")
    T.matmul(scp, C["wpool_stat"], h1, start=True, stop=True)
    scs = nodep.tile([8, N], F32, name=f"scs_{rep}", tag="scs")
    S.activation(scs, scp, Act.Copy, bias=0.0, scale=1.0)
    dbg("score", scs)
    snode = nodep.tile([128, 4, 8], F32, name=f"snode_{rep}", tag="snode")
    for t in range(4):
        tp = psum1.tile([128, 8], F32, name="tp", tag="small")
        T.transpose(tp, scs[:, 128 * t:128 * (t + 1)], C["identT"][0:8, 0:8])
        S.activation(snode[:, t, :], tp, Act.Copy, bias=0.0, scale=1.0)
    sneg = nodep.tile([128, 4, 8], F32, name=f"sneg_{rep}", tag="sneg")
    V.tensor_scalar(sneg, snode, -1.0, None, Alu.mult)
    # rank: graphs 0-3 on Act (sign-sum), graphs 4-7 on DVE (is_gt count)
    rk = nodep.tile([128, 4, 8], F32, name=f"rk_{rep}", tag="rk")
    # interleave Act-half (g<4) and DVE-half (g>=4) so both engines run
    # concurrently despite the 2-deep sb psum ring
    for g in (0, 4, 1, 5, 2, 6, 3, 7):
        sb = psum.tile([128, N], F32, name="sb", tag="zc")
        T.matmul(sb, C["onesel_stat"][:, 128 * g:128 * (g + 1)],
                 scs, start=True, stop=True)
        if g < 4:
            for t in range(4):
                sga = work.tile([128, N], F16, name="sga", tag="sga")
                S.activation(sga, sb, Act.Sign, bias=sneg[:, t, g:g + 1],
                             scale=1.0, accum_out=rk[:, t, g:g + 1])
        else:
            for t in range(4):
                sgv = work.tile([128, N], F16, name="sgv", tag="sgv")
                V.tensor_scalar(sgv, sb, snode[:, t, g:g + 1], 0.0,
                                Alu.is_gt, Alu.add,
                                accum_out=rk[:, t, g:g + 1])
    dbg("rk", rk.rearrange("p a b -> p (a b)"))
    if STOP_STAGE == "rank":
        return stop_dma(rk[0:4, 0, 0:G])
    mask01 = nodep.tile([128, 4, 8], F32, name=f"mask01_{rep}", tag="mask01")
    V.tensor_scalar(mask01[:, :, 0:4], rk[:, :, 0:4], -1.0, None, Alu.is_le)
    V.tensor_scalar(mask01[:, :, 4:8], rk[:, :, 4:8], float(K) - 0.5, None,
                    Alu.is_le)
    dbg("mask", mask01.rearrange("p a b -> p (a b)"))
    ex = nodep.tile([128, 4, 8], F32, name=f"ex_{rep}", tag="ex")
    S.activation(ex, snode, Act.Exp, bias=0.0, scale=-2.0)
    exd = nodep.tile([128, 4, 8], F32, name=f"exd_{rep}", tag="exd")
    V.tensor_scalar(exd, ex, 1.0, None, Alu.add)
    exr = nodep.tile([128, 4, 8], F32, name=f"exr_{rep}", tag="exr")
    V.reciprocal(exr, exd)
    th = nodep.tile([128, 4, 8], F32, name=f"th_{rep}", tag="th")
    V.tensor_scalar(th, exr, 2.0, -1.0, Alu.mult, Alu.add)
    gate = nodep.tile([128, 4, 8], F32, name=f"gate_{rep}", tag="gate")
    V.tensor_tensor(out=gate, in0=th, in1=mask01, op=Alu.mult)
    gq = nodep.tile([128, 4, 8], F32, name=f"gq_{rep}", tag="gq")
    V.tensor_scalar(gq, mask01, -1.0, BIGNEG, Alu.add, Alu.mult)
    gqm = nodep.tile([72, N], F16, name=f"gqm_{rep}", tag="gqm")
    gfm, qfm, mfm = gqm[0:8, :], gqm[32:40, :], gqm[64:72, :]
    for t in range(4):
        tstack = work.tile([128, 72], F32, name="tstack", tag="tstack")
        V.memset(tstack[:, 8:32], 0.0)
        V.memset(tstack[:, 40:64], 0.0)
        V.tensor_copy(out=tstack[:, 0:8], in_=gate[:, t, :])
        V.tensor_copy(out=tstack[:, 32:40], in_=gq[:, t, :])
        V.tensor_copy(out=tstack[:, 64:72], in_=mask01[:, t, :])
        tq = psum1.tile([72, 128], F32, name="tq", tag="small")
        T.transpose(tq, tstack, C["identT"])
        S.activation(gqm[:, 128 * t:128 * (t + 1)], tq, Act.Copy,
                     bias=0.0, scale=1.0)
    # masked-src edge counts: mmg[g, n] = sum_m (mask_g[m]-1) * A_g[m, n];
    # per (g, block) lhsT is [128, 8] with only column g nonzero, so all 32
    # matmuls accumulate into one [8, N] psum (zero rows elsewhere)
    mfm1 = nodep.tile([8, N], F32, name=f"mfm1_{rep}", tag="mfm1")
    V.tensor_scalar(mfm1, mfm, -1.0, None, Alu.add)
    zsb = nodep.tile([128, 4, 64], F16, name=f"zsb_{rep}", tag="zsb")
    for b in range(4):
        zp = psum1.tile([128, 64], F32, name="zp", tag="small")
        T.matmul(zp, mfm1[:, 128 * b:128 * (b + 1)], C["d8_stat"],
                 start=True, stop=True)
        S.activation(zsb[:, b, :], zp, Act.Copy, bias=0.0, scale=1.0)
    mmg_ps = psumw.tile([8, N], F32, name="mmg_ps", tag="wide")
    for g in range(G):
        if g == 4:   # second half of A overwrites the buffer (WAR-tracked)
            nc.sync.dma_start(out=C["amat"], in_=C["amat_dram"][:, 16 * 512:])
        for b in range(4):
            T.matmul(mmg_ps, zsb[:, b, 8 * g:8 * g + 8],
                     C["amat"][:, ((g % 4) * 4 + b) * 512:
                               ((g % 4) * 4 + b + 1) * 512],
                     start=(g == 0 and b == 0), stop=(g == 7 and b == 3))
    mmg_s = nodep.tile([8, N], F16, name=f"mmg_s_{rep}", tag="mmg_s")
    S.activation(mmg_s, mmg_ps, Act.Copy, bias=0.0, scale=1.0 / DEG)

    gb = psume.tile([128, N], F32, name="gb", tag="e512")
    T.matmul(gb, C["ones16b_stat"], gfm, start=True, stop=True)
    hp = nodep.tile([128, N], F32, name=f"hp_{rep}", tag="hp")
    V.tensor_tensor(out=hp, in0=h1, in1=gb, op=Alu.mult)
    dbg("hp", hp)
    qb = psume.tile([128, N], F32, name="qb", tag="e512")
    T.matmul(qb, C["ones16b32_stat"][32:40, :], qfm,
             start=True, stop=True)
    hq = nodep.tile([128, NT, 1], F32, name=f"hq_{rep}", tag="hq")
    V.tensor_tensor(out=hq[:, :N, 0], in0=hp, in1=qb, op=Alu.add)
    S.activation(hq[:, N:, 0], qb[:, 0:NT - N], Act.Copy, bias=0.0, scale=0.0)
    dbg("hq", hq[:, :N, 0])
    if STOP_STAGE == "hq":
        return stop_dma(hq[0:4, 0:G, 0].bitcast(F32))

    # ================= Layer 2 =================
    h2 = [nodep.tile([128, N], F16, name=f"h2{sl}_{rep}", tag=f"h2{sl}")
          for sl in range(2)]
    emit2 = half_pipe(1, hp, mmg_s, C["w2a_stat"], C["b2a_vec"],
                      C["g2_vec"], C["be2_vec"], C["w2bh_stat"], h2)
    E2 = edge_layer(1, hq, C["t2vec"], C["bee_vec"], mm=mmg_s)
    if STOP_STAGE == "l2agg":
        return stop_dma(mmg_s[0:4, 0:G])
    emit2(E2, 0, HALVES[0])
    dbg("h2a", h2[0])
    dbg("h2b", h2[1])

    # ================= pooling + head =================
    pooled = []
    for sl, statname in ((0, "maskbc_statA"), (1, "maskbc_statB")):
        mb2 = psume.tile([128, N], F32, name=f"mbp{sl}", tag="e512")
        T.matmul(mb2, C[statname][64:72, :], mfm, start=True, stop=True)
        mbh = nodep.tile([128, N], F16, name=f"mbh{sl}_{rep}", tag=f"mbh{sl}")
        S.activation(mbh, mb2, Act.Copy, bias=0.0, scale=1.0)
        pl = nodep.tile([128, 1], F32, name=f"pl{sl}_{rep}", tag=f"pl{sl}")
        scratch = work.tile([128, N], F16, name="plscratch", tag="plscratch",
                            bufs=1)
        V.scalar_tensor_tensor(out=scratch, in0=h2[sl], scalar=1.0, in1=mbh,
                               op0=Alu.mult, op1=Alu.mult, accum_out=pl)
        pooled.append(pl)
    P8 = psum1.tile([32, G], F32, name="P8", tag="small")
    for g in range(G):
        sl, gg = g // 4, g % 4
        T.matmul(P8[:, g:g + 1],
                 C["selk_stat"][:, 32 * gg:32 * gg + 32],
                 pooled[sl], start=True, stop=True,
                 skip_group_check=True)
    p8s = nodep.tile([32, G], F32, name=f"p8s_{rep}", tag="p8s")
    S.activation(p8s, P8, Act.Copy, bias=0.0, scale=1.0)
    dbg("pooled", p8s)
    a1p = psume.tile([128, G], F32, name="a1p", tag="e512")
    T.matmul(a1p, C["wa_stat"], p8s, start=True, stop=True)
    a1 = nodep.tile([128, G], F32, name=f"a1_{rep}", tag="a1")
    S.activation(a1, a1p, Act.Relu, bias=C["ba_vec"], scale=1.0)
    op = psum1.tile([4, G], F32, name="op", tag="small")
    T.matmul(op, C["wo_stat"], a1, start=True, stop=True)
    oe = nodep.tile([4, G], F32, name=f"oe_{rep}", tag="oe")
    S.activation(oe, op, Act.Exp, bias=C["bo2_vec"], scale=-2.0)
    od = nodep.tile([4, G], F32, name=f"od_{rep}", tag="od")
    V.tensor_scalar(od, oe, 1.0, None, Alu.add)
    orr = nodep.tile([4, G], F32, name=f"orr_{rep}", tag="orr")
    V.reciprocal(orr, od)
    ot = nodep.tile([4, G], F32, name=f"ot_{rep}", tag="ot")
    V.tensor_scalar(ot, orr, 2.0, -1.0, Alu.mult, Alu.add)
    nc.sync.dma_start(out=out_ext, in_=ot)


# ----------------------------------------------------------------------------
# Self-contained entry point: kernel(**inputs) -> [64, 4] float32
# ----------------------------------------------------------------------------
import jax as _jax
from jax.sharding import Mesh as _Mesh, PartitionSpec as _PartitionSpec
from jax.experimental.shard_map import shard_map as _shard_map

_COMPILED = {}


def _build_and_jit():
    """Re-create the jitted executable on every call: re-executing a loaded
    NEFF leaves device state (semaphores) behind and corrupts the second run,
    so each kernel() invocation gets a fresh executable (BIR->NEFF is
    disk-cached, so this costs seconds, not a recompile)."""
    from concourse import bass2jax
    from concourse.bass2jax import _bass_exec_p, partition_id_tensor

    if "nc" in _COMPILED:
        nc = _COMPILED["nc"]
    else:
        nc = build_nc()
        _COMPILED["nc"] = nc
    bass2jax.install_neuronx_cc_hook()
    partition_name = (nc.partition_id_tensor.name
                      if nc.partition_id_tensor else None)
    in_names, out_names, out_avals, zero_outs = [], [], [], []
    for alloc in nc.m.functions[0].allocations:
        if not isinstance(alloc, mybir.MemoryLocationSet):
            continue
        nm = alloc.memorylocations[0].name
        if alloc.kind == "ExternalInput":
            if nm != partition_name:
                in_names.append(nm)
        elif alloc.kind == "ExternalOutput":
            out_names.append(nm)
            out_avals.append(_jax.core.ShapedArray(
                tuple(alloc.tensor_shape), mybir.dt.np(alloc.dtype)))
            zero_outs.append(np.zeros(tuple(alloc.tensor_shape),
                                      mybir.dt.np(alloc.dtype)))
    n_params = len(in_names)
    n_outs = len(out_avals)
    in_names_all = in_names + out_names
    if partition_name is not None:
        in_names_all.append(partition_name)
    donate = tuple(range(n_params, n_params + n_outs))

    def _body(*args):
        operands = list(args)
        if partition_name is not None:
            operands.append(partition_id_tensor())
        return tuple(_bass_exec_p.bind(
            *operands, out_avals=tuple(out_avals),
            in_names=tuple(in_names_all), out_names=tuple(out_names),
            lowering_input_output_aliases=(), sim_require_finite=True,
            sim_require_nnan=True, nc=nc))

    devices = _jax.devices()[:8]
    mesh = _Mesh(np.asarray(devices), ("core",))
    in_specs = (_PartitionSpec("core"),) * (n_params + n_outs)
    out_specs = (_PartitionSpec("core"),) * len(out_names)
    sharded = _jax.jit(
        _shard_map(_body, mesh=mesh, in_specs=in_specs, out_specs=out_specs,
                   check_rep=False),
        donate_argnums=donate, keep_unused=True)
    return (sharded, in_names, out_names, zero_outs)


def kernel(**inputs):
    """Full-input GNN forward on 8 TRN2 NeuronCores; returns [64, 4] f32."""
    sharded, in_names, out_names, zero_outs = _build_and_jit()
    core_maps = prep_inputs(inputs)
    concat_in = [np.concatenate([core_maps[c][nm] for c in range(8)], axis=0)
                 for nm in in_names]
    concat_zero = [np.zeros((8 * z.shape[0], *z.shape[1:]), z.dtype)
                   for z in zero_outs]
    out_arrs = sharded(*concat_in, *concat_zero)
    oi = out_names.index("out")
    full = np.asarray(out_arrs[oi]).reshape(8, 4, G)
    return np.concatenate([full[c].T for c in range(8)], axis=0)


# revision 103
# speedup vs baseline: 1.0646x; 1.0182x over previous
"""GNN (GENConv x2 + TopK pool) Bass/Tile kernel for TRN2, data-parallel over
8 NeuronCores (8 graphs per core).

Edge aggregation uses a fixed-degree main layout: node n owns edge slots
[32n, 32n+32) (ghost-padded, corrected analytically via host-known pad
counts), so the scatter-softmax segment sums become strided windowed
tensor_reduce ops -- no prefix-scan carry chains, no big boundary gathers.
Overflow edges (deg > 32, ~7%) go to a 2048-slot dst-sorted spill region per
graph handled by a small scan + boundary gather.

Per-core layout ("fm" = feature-major packed): edge tensors are
[128 = 16feat x 8graph, 18432 slots]; node tensors [128, 512].
"""

import numpy as np
from contextlib import ExitStack

import concourse.bass as bass
import concourse.bacc as bacc
import concourse.mybir as mybir
import concourse.tile as tile
from concourse import library_config

F32 = mybir.dt.float32
F32R = mybir.dt.float32r
F16 = mybir.dt.float16
F8E4 = mybir.dt.float8e4
I16 = mybir.dt.int16
I8 = mybir.dt.int8

G = 8          # graphs per core
N = 512        # nodes per graph
DEG = 32       # fixed main-slot degree per node
EG = 16384     # edges per graph (input)
EM = N * DEG   # main slots per graph = 16384
ESP = 1536     # spill slots per graph (max observed 1248)
ET = EM + ESP  # total slots = 18432
EF = 16        # edge/node feature dim after encode
XF = 64        # input node feature dim
K = 256        # topk keep
CHUNK = 1024   # slots per chunk
NCH = ET // CHUNK        # 18 chunks (16 main + 2 spill)
NMAIN = EM // CHUNK      # 16
NT = 528       # gather table columns (512 nodes + ghost pad; 16-multiple)
GHOST = 512
NEND = 576     # padded spill end-list length (513 used)
EPS = 1e-7
BIGNEG = 6.0e4
LN32 = float(np.log(DEG))

import os as _os
STOP_STAGE = _os.environ.get("K_STOP_STAGE") or None  # hw bisect hook


# ----------------------------------------------------------------------------
# Host-side preprocessing: full inputs -> per-core named arrays
# ----------------------------------------------------------------------------

def prep_inputs(inputs: dict) -> list[dict]:
    x = np.asarray(inputs["x"], np.float32)            # [B*N, 64]
    ei = np.asarray(inputs["edge_index"])              # [2, E] int64
    ea = np.asarray(inputs["edge_attr"], np.float32)   # [E, 16]
    B = 64
    assert x.shape == (B * N, XF)
    assert ea.shape == (B * EG, EF)

    src_g = (ei[0] % N).astype(np.int64)
    dst_g = (ei[1] % N).astype(np.int64)
    graph_of_edge = (ei[0] // N).astype(np.int64)
    assert np.array_equal(graph_of_edge, np.repeat(np.arange(B), EG)), \
        "edge blocks not per-graph; prep assumes reference setup_inputs layout"
    assert np.array_equal(ei[0] // N, ei[1] // N)

    def lin(name):
        return np.asarray(inputs[name], np.float32)

    W_ne, b_ne = lin("W_ne"), lin("b_ne")
    W_ee, b_ee = lin("W_ee"), lin("b_ee")
    W1a, b1a, g1, be1 = lin("W1a"), lin("b1a"), lin("g1"), lin("be1")
    W1b, b1b = lin("W1b"), lin("b1b")
    W2a, b2a, g2, be2 = lin("W2a"), lin("b2a"), lin("g2"), lin("be2")
    W2b, b2b = lin("W2b"), lin("b2b")
    Wa, ba, Wo, bo = lin("Wa"), lin("ba"), lin("Wo"), lin("bo")
    w_pool = lin("w_pool")
    wp = w_pool / np.linalg.norm(w_pool)
    t1 = np.float32(inputs["t1"])
    t2 = np.float32(inputs["t2"])

    # centering fold: LN(y) uses yC = y - mean(y) = u @ (W C) + b C
    C32 = np.eye(32, dtype=np.float32) - 1.0 / 32.0
    W1aC = W1a @ C32
    b1aC = b1a @ C32
    W2aC = W2a @ C32
    b2aC = b2a @ C32

    cst = {}
    wne = np.zeros((128, 4 * 128), np.float32)
    for p in range(4):
        for a in range(2):
            gg = 2 * p + a
            wne[64 * a:64 * a + XF, 128 * p + 16 * gg:128 * p + 16 * gg + EF] = W_ne
    cst["wne_stat"] = wne.astype(np.float16)
    cst["bne_vec"] = np.tile(b_ne, G)[:, None].astype(np.float32)
    wee = np.zeros((128, 128), np.float32)
    for g in range(G):
        wee[16 * g:16 * g + EF, 16 * g:16 * g + EF] = W_ee
    cst["wee_stat"] = wee.astype(np.float16)
    cst["bee_vec"] = np.tile(b_ee, G)[:, None].astype(np.float32)
    cst["mln32_vec"] = np.full((128, 1), -LN32, np.float32)
    cst["identT"] = np.eye(128, dtype=np.float32)
    cst["ident"] = np.eye(128, dtype=np.float32)
    cst["t1vec"] = np.full((128, 1), t1, np.float32)
    cst["t2vec"] = np.full((128, 1), t2, np.float32)
    w1a = np.zeros((64, 128), np.float32)
    for gg in range(4):
        w1a[16 * gg:16 * gg + 16, 32 * gg:32 * gg + 32] = W1aC
    cst["w1a_stat"] = np.vstack([w1a, w1a])
    cst["b1a_vec"] = np.tile(b1aC, 4)[:, None].astype(np.float32)
    ones32 = np.zeros((128, 4), np.float32)
    for gg in range(4):
        ones32[32 * gg:32 * gg + 32, gg] = 1.0 / 32.0
    cst["ones32h_stat"] = ones32.astype(np.float16)
    onesb32 = np.zeros((4, 128), np.float32)
    for gg in range(4):
        onesb32[gg, 32 * gg:32 * gg + 32] = 1.0
    cst["onesb32_stat"] = onesb32
    cst["g1_vec"] = np.tile(g1, 4)[:, None].astype(np.float32)
    cst["be1_vec"] = np.tile(be1, 4)[:, None].astype(np.float32)
    w1b = np.zeros((64, 32), np.float32)
    for gg in range(2):
        w1b[32 * gg:32 * gg + 32, 16 * gg:16 * gg + 16] = W1b
    cst["w1bh_stat"] = np.vstack([w1b, w1b]).astype(np.float16)
    cst["b1b_vec"] = np.tile(b1b, G)[:, None].astype(np.float32)
    wpool = np.zeros((128, 8), np.float32)
    for g in range(G):
        wpool[16 * g:16 * g + EF, g] = wp
    cst["wpool_stat"] = wpool
    ones16b = np.zeros((8, 128), np.float32)
    for g in range(G):
        ones16b[g, 16 * g:16 * g + EF] = 1.0
    cst["ones16b_stat"] = ones16b.astype(np.float16)
    o16s = np.zeros((40, 128), np.float32)
    o16s[32:40] = ones16b
    cst["ones16b32_stat"] = o16s.astype(np.float16)
    onesel = np.zeros((8, 8 * 128), np.float32)
    for g in range(8):
        onesel[g, 128 * g:128 * (g + 1)] = 1.0
    cst["onesel_stat"] = onesel
    w2a = np.zeros((64, 128), np.float32)
    for gg in range(4):
        w2a[16 * gg:16 * gg + 16, 32 * gg:32 * gg + 32] = W2aC
    cst["w2a_stat"] = np.vstack([w2a, w2a])
    cst["b2a_vec"] = np.tile(b2aC, 4)[:, None].astype(np.float32)
    cst["g2_vec"] = np.tile(g2, 4)[:, None].astype(np.float32)
    cst["be2_vec"] = np.tile(be2, 4)[:, None].astype(np.float32)
    w2b = np.zeros((64, 64), np.float32)
    for gg in range(2):
        w2b[32 * gg:32 * gg + 32, 32 * gg:32 * gg + 32] = W2b
    cst["w2bh_stat"] = np.vstack([w2b, w2b]).astype(np.float16)
    cst["b2b_vec"] = np.tile(b2b, 4)[:, None].astype(np.float32)
    mbA = np.zeros((8, 128), np.float32)
    mbB = np.zeros((8, 128), np.float32)
    for g in range(4):
        mbA[g, 32 * g:32 * g + 32] = 1.0
        mbB[g + 4, 32 * g:32 * g + 32] = 1.0
    mbA64 = np.zeros((72, 128), np.float32); mbA64[64:72] = mbA
    mbB64 = np.zeros((72, 128), np.float32); mbB64[64:72] = mbB
    cst["maskbc_statA"] = mbA64.astype(np.float16)
    cst["maskbc_statB"] = mbB64.astype(np.float16)
    selk = np.zeros((128, 4 * 32), np.float32)
    for gg in range(4):
        selk[32 * gg:32 * gg + 32, 32 * gg:32 * gg + 32] = np.eye(32) / K
    cst["selk_stat"] = selk
    d8 = np.zeros((8, 64), np.float32)
    for g in range(G):
        d8[g, 9 * g] = 1.0   # spreads transpose(mask) onto per-graph columns
    cst["d8_stat"] = d8
    cst["wa_stat"] = Wa.astype(np.float32)
    cst["ba_vec"] = ba[:, None].astype(np.float32)
    cst["wo_stat"] = Wo.astype(np.float32)
    cst["bo2_vec"] = (-2.0 * bo)[:, None].astype(np.float32)
    cst["lneps_vec"] = np.full((4, 1), 1e-5, np.float32)

    core_maps = []
    for core in range(8):
        m = dict(cst)
        gsl = slice(core * G, (core + 1) * G)
        xt = np.zeros((128, 4 * 512), np.float32)
        xs = x.reshape(B, N, XF)[gsl]
        for p in range(4):
            for a in range(2):
                xt[64 * a:64 * a + XF, 512 * p:512 * (p + 1)] = xs[2 * p + a].T
        m["xT"] = xt.astype(np.float16)

        attrT = np.zeros((128, ET), np.float16)
        srcidx = np.zeros((128, ET // 16), np.int16)
        spendidx = np.zeros((128, NEND // 16), np.int16)
        padcnt = np.zeros((128, N), np.float16)
        amat = np.zeros((128, 32 * 512), np.float16)
        for gl in range(G):
            gid = core * G + gl
            s_l = src_g[gid * EG:(gid + 1) * EG]
            d_l = dst_g[gid * EG:(gid + 1) * EG]
            order = np.argsort(d_l, kind="stable")
            ds = d_l[order]
            ss = s_l[order]
            ats = ea[gid * EG:(gid + 1) * EG][order]     # [EG, 16] dst-sorted
            counts = np.bincount(ds, minlength=N)
            starts = np.zeros(N + 1, np.int64)
            np.cumsum(counts, out=starts[1:])
            # main: node n gets its first min(deg,32) edges at slots 32n+o
            j = np.arange(EM)
            nn = j // DEG
            oo = j % DEG
            msk = oo < np.minimum(counts[nn], DEG)
            pos = starts[nn] + oo
            srcf = np.full(ET, GHOST, np.int64)
            attrf = np.zeros((ET, EF), np.float32)
            srcf[j[msk]] = ss[pos[msk]]
            attrf[j[msk]] = ats[pos[msk]]
            # spill: rank within dst-run >= 32 (already dst-sorted)
            r = np.arange(EG) - starts[ds]
            spm = r >= DEG
            nsp = int(spm.sum())
            assert nsp <= ESP, f"spill overflow: {nsp} > {ESP}"
            srcf[EM:EM + nsp] = ss[spm]
            attrf[EM:EM + nsp] = ats[spm]
            spd = ds[spm]
            e_sp = np.searchsorted(spd, np.arange(N), side="right")
            elist = np.zeros(NEND, np.int16)
            elist[1:N + 1] = e_sp.astype(np.int16)

            attrT[16 * gl:16 * gl + EF, :] = attrf.T.astype(np.float16)
            srcidx[16 * gl:16 * gl + 16, :] = \
                srcf.astype(np.int16).reshape(ET // 16, 16).T
            spendidx[16 * gl:16 * gl + 16, :] = \
                elist.reshape(NEND // 16, 16).T
            padcnt[16 * gl:16 * gl + 16, :] = \
                np.maximum(0, DEG - counts)[None, :].astype(np.float16)
            # adjacency count matrix A[m, n] = #edges m->n (for the L2
            # masked-src denominator correction via matmul)
            A = np.zeros((N, N), np.int32)
            np.add.at(A, (s_l, d_l), 1)
            assert A.max() <= 2048, A.max()  # f16 integers exact to 2048
            for b in range(4):
                amat[:, (4 * gl + b) * 512:(4 * gl + b + 1) * 512] = \
                    A[128 * b:128 * (b + 1), :].astype(np.float16)
        m["spendidx"] = spendidx
        m["padcnt"] = padcnt
        blob = np.zeros((128, CBLOB_BYTES), np.uint8)
        for name, shape, dt, off in CONST_SPECS:
            arr = m[name]
            bv = arr.view(np.uint8).reshape(arr.shape[0], -1)
            blob[:arr.shape[0], off:off + bv.shape[1]] = bv
        rblob = np.zeros((128, 4 * 128), np.float32)
        rblob[:, 0:128] = m["ident"]
        rblob[:, 128:256] = m["w1a_stat"]
        rblob[:, 256:384] = m["w2a_stat"]
        rblob[0:4, 384:512] = m["onesb32_stat"]
        core_maps.append({"cblob": blob, "rblob": rblob, "attrT": attrT,
                          "xT": m["xT"], "srcidx": srcidx, "amat": amat})
    return core_maps


_CSPEC_RAW = [
    # encode-critical constants first (covered by the first cblob DMA piece)
    ("wne_stat", [128, 4 * 128], F16),
    ("bne_vec", [128, 1], F32),
    ("wee_stat", [128, 128], F16),
    ("bee_vec", [128, 1], F32),
    ("mln32_vec", [128, 1], F32),
    ("t1vec", [128, 1], F32),
    ("t2vec", [128, 1], F32),
    ("spendidx", [128, NEND // 16], I16),
    ("padcnt", [128, N], F16),
    # ---- split point: everything below arrives with the second DMA ----
    ("identT", [128, 128], F32),
    ("b1a_vec", [128, 1], F32),
    ("ones32h_stat", [128, 4], F16),
    ("g1_vec", [128, 1], F32),
    ("be1_vec", [128, 1], F32),
    ("w1bh_stat", [128, 32], F16),
    ("b1b_vec", [128, 1], F32),
    ("wpool_stat", [128, 8], F32),
    ("ones16b_stat", [8, 128], F16),
    ("ones16b32_stat", [40, 128], F16),
    ("onesel_stat", [8, 8 * 128], F32),
    ("b2a_vec", [128, 1], F32),
    ("g2_vec", [128, 1], F32),
    ("be2_vec", [128, 1], F32),
    ("w2bh_stat", [128, 64], F16),
    ("b2b_vec", [128, 1], F32),
    ("maskbc_statA", [72, 128], F16),
    ("maskbc_statB", [72, 128], F16),
    ("selk_stat", [128, 4 * 32], F32),
    ("d8_stat", [8, 64], F32),
    ("wa_stat", [32, 128], F32),
    ("ba_vec", [128, 1], F32),
    ("wo_stat", [128, 4], F32),
    ("bo2_vec", [4, 1], F32),
    ("lneps_vec", [4, 1], F32),
]

_SPLIT_AFTER = "padcnt"


def _mk_const_specs():
    specs = []
    off = 0
    for nm, shape, dt in _CSPEC_RAW:
        nbytes = int(np.prod(shape[1:])) * mybir.dt.size(dt)
        specs.append((nm, shape, dt, off))
        off += (nbytes + 127) // 128 * 128
    return specs, off

CONST_SPECS, CBLOB_BYTES = _mk_const_specs()

INPUT_SPECS = [
    ("cblob", [128, CBLOB_BYTES], mybir.dt.uint8),
    ("rblob", [128, 4 * 128], F32R),   # ident | w1a | w2a | onesb32(rows 0:4)
    ("attrT", [128, ET], F16),
    ("xT", [128, 4 * 512], F16),
    ("srcidx", [128, ET // 16], I16),
    ("amat", [128, 32 * 512], F16),
]


# ----------------------------------------------------------------------------
# Device graph
# ----------------------------------------------------------------------------

def build_nc(debug_keys=(), n_rep=1):
    nc = bacc.Bacc(None, target_bir_lowering=False, debug=False)
    A = {}
    for name, shape, dt in INPUT_SPECS:
        A[name] = nc.declare_dram_parameter(name, shape, dt, isOutput=False)[:]
    out_ext = nc.declare_dram_parameter("out", [4, G], F32, isOutput=True)[:]
    dbg_ext = {}
    dbg_shapes = {
        "hT": ((128, N), F32), "mc0": ((128, CHUNK), F16),
        "ev0": ((128, CHUNK), F16), "evm0": ((128, CHUNK), F16),
        "sums0": ((128, N), F16), "sums1": ((128, N), F16),
        "spev0": ((128, ESP), F16), "Ssp0": ((128, ESP + 1), F32),
        "Dsp0": ((128, N), F32), "Dsp1": ((128, N), F32),
        "aggr1": ((128, N), F32), "u1": ((128, N), F32),
        "h1": ((128, N), F32),
        "score": ((8, N), F32), "rk": ((128, 32), F32),
        "mask": ((128, 32), F32),
        "hq": ((128, N), F32), "hp": ((128, N), F32),
        "aggr2": ((128, N), F32), "h2a": ((128, N), F16),
        "h2b": ((128, N), F16), "pooled": ((32, 8), F32),
        "y1sA": ((128, N), F16), "r1A": ((128, N), F16),
    }
    for key in debug_keys:
        shape, dt = dbg_shapes[key]
        dbg_ext[key] = nc.declare_dram_parameter(
            "dbg_" + key, list(shape), dt, isOutput=True)[:]

    with tile.TileContext(nc) as tc, ExitStack() as ctx:
        consts = ctx.enter_context(tc.tile_pool(name="consts", bufs=1))
        nodep = ctx.enter_context(tc.tile_pool(name="nodep", bufs=1))
        work = ctx.enter_context(tc.tile_pool(name="work", bufs=2))
        psum = ctx.enter_context(tc.tile_pool(name="psum", bufs=2, space="PSUM"))
        psum1 = ctx.enter_context(tc.tile_pool(name="psum1", bufs=1, space="PSUM"))
        psume = ctx.enter_context(tc.tile_pool(name="psume", bufs=1, space="PSUM"))
        psumw = ctx.enter_context(tc.tile_pool(name="psumw", bufs=1, space="PSUM"))

        nc.gpsimd.load_library(library_config.ap_gather)

        # ---- DMAs: xT first (encode-critical), then cblob, srcidx, attr ----
        xts = consts.tile([128, 4 * 512], F16, name="xT_sb")
        nc.sync.dma_start(out=xts, in_=A["xT"])
        cb = consts.tile([128, CBLOB_BYTES], mybir.dt.uint8, name="cblob_sb")
        _split = next(off for nm, _s, _d, off in CONST_SPECS
                      if nm == "identT")
        nc.sync.dma_start(out=cb[:, :_split], in_=A["cblob"][:, :_split])
        srcidx_sb = consts.tile([128, ET // 16], I16, name="srcidx_sb")
        nc.sync.dma_start(out=srcidx_sb, in_=A["srcidx"])
        attr_all = consts.tile([128, ET], F16, name="attr_sb")
        # pieces in chunk-consumption order: main[0:2048], spill, rest
        nc.sync.dma_start(out=attr_all[:, 0:2048], in_=A["attrT"][:, 0:2048])
        rb_sb = consts.tile([128, 4 * 128], F32R, name="rblob_sb")
        nc.sync.dma_start(out=rb_sb, in_=A["rblob"])
        nc.sync.dma_start(out=attr_all[:, EM:], in_=A["attrT"][:, EM:])
        for lo, hi in ((2048, 8192), (8192, 12288), (12288, EM)):
            nc.sync.dma_start(out=attr_all[:, lo:hi], in_=A["attrT"][:, lo:hi])
        nc.sync.dma_start(out=cb[:, _split:], in_=A["cblob"][:, _split:])
        amat_sb = consts.tile([128, 16 * 512], F16, name="amat_sb")
        nc.sync.dma_start(out=amat_sb, in_=A["amat"][:, :16 * 512])

        C = {"srcidx": srcidx_sb, "attr": attr_all, "amat": amat_sb,
             "amat_dram": A["amat"],
             "ident": rb_sb[:, 0:128], "w1a_stat": rb_sb[:, 128:256],
             "w2a_stat": rb_sb[:, 256:384],
             "onesb32_stat": rb_sb[0:4, 384:512]}
        for name, shape, dt, off in CONST_SPECS:
            nbytes = int(np.prod(shape[1:])) * mybir.dt.size(dt)
            ap = cb[:shape[0], off:off + nbytes].bitcast(dt)
            if len(shape) == 3:
                ap = ap.rearrange("p (a b) -> p a b", b=shape[2])
            C[name] = ap

        dbg_keys_set = set(debug_keys)

        def dbg(key, ap):
            if key in dbg_keys_set:
                nc.sync.dma_start(out=dbg_ext[key], in_=ap)

        for rep in range(n_rep):
            run_once(nc, tc, A, C, xts, out_ext, dbg, nodep, work,
                     psum, psum1, psume, psumw, rep)
    nc.compile()
    return nc


def run_once(nc, tc, A, C, xts, out_ext, dbg, nodep, work, psum, psum1,
             psume, psumw, rep):
    V = nc.vector
    S = nc.scalar
    T = nc.tensor
    Alu = mybir.AluOpType
    Act = mybir.ActivationFunctionType

    # ================= node encode: hT = x @ W_ne + b_ne =================
    hraw = psume.tile([128, N], F32, name="hraw", tag="e512")
    for p in range(4):
        T.matmul(hraw, C["wne_stat"][:, 128 * p:128 * (p + 1)],
                 xts[:, 512 * p:512 * (p + 1)],
                 start=(p == 0), stop=(p == 3))
    hT = nodep.tile([128, NT, 1], F32, name=f"hT_{rep}")
    S.activation(hT[:, :N, 0], hraw, Act.Identity, bias=C["bne_vec"], scale=1.0)
    S.activation(hT[:, N:, 0], hraw[:, 0:NT - N], Act.Copy, bias=0.0, scale=0.0)
    dbg("hT", hT[:, :N, 0])

    def stop_dma(ap):
        nc.sync.dma_start(out=out_ext, in_=ap)

    if STOP_STAGE == "enc":
        return stop_dma(hT[0:4, 0:G, 0].bitcast(F32))

    # ================= shared edge-phase machinery =================
    def edge_layer(layer, table, tvec, bee, mm=None, dbg_on=False,
                   inject=None):
        """Returns aggr tile [128, N] f32 (softmax-weighted mean + pad fix)."""
        sums = nodep.tile([128, 2, N], F32, name=f"sums_{layer}_{rep}",
                          tag="sums")
        spev = nodep.tile([128, 2, ESP], F16, name=f"spev_{layer}_{rep}",
                          tag="spev")
        # ev for ghost-pad slots: mc_pad = relu(bee); ev_pad = exp(t*mc - ln32)
        beer = nodep.tile([128, 1], F32, name=f"beer_{layer}_{rep}", tag="beer")
        S.activation(beer, bee, Act.Relu, bias=0.0, scale=1.0)
        evpad = nodep.tile([128, 1], F32, name=f"evpad_{layer}_{rep}",
                           tag="evpad")
        S.activation(evpad, beer, Act.Exp, bias=C["mln32_vec"], scale=tvec)
        evmpad = nodep.tile([128, 1], F32, name=f"evmpad_{layer}_{rep}",
                            tag="evmpad")
        V.tensor_tensor(out=evmpad, in0=evpad, in1=beer, op=Alu.mult)

        # 2 main chunks prime the pipeline, then spill (so its scan +
        # boundary-gather tail overlaps the remaining main chunks)
        E = (sums, None, evpad, evmpad)
        for cc in [0, 1, NMAIN, NMAIN + 1] + list(range(2, NMAIN)):
            if inject is not None and cc == inject[0]:
                E = (sums, Dsp, evpad, evmpad)
                inject[1](E)
            base = cc * CHUNK
            csz = CHUNK if cc != NMAIN + 1 else ESP - CHUNK
            hsrc = work.tile([128, CHUNK, 1], F32, name="hsrc", tag="hsrc", bufs=3)
            j0 = base // 16
            nc.gpsimd.ap_gather(
                hsrc[:, :csz, :], table, C["srcidx"][:, j0:j0 + csz // 16],
                channels=128, num_elems=NT, d=1, num_idxs=csz)
            if STOP_STAGE == "pipe1a" and layer == 0:
                return None
            zc = psum.tile([128, CHUNK], F32, name="zc", tag="zc")
            for s in range(csz // 512):
                sl = slice(512 * s, 512 * (s + 1))
                T.matmul(zc[:, sl], C["wee_stat"],
                         C["attr"][:, base + 512 * s:base + 512 * (s + 1)],
                         start=True, stop=False, skip_group_check=True)
                T.matmul(zc[:, sl], C["ident"].bitcast(F32), hsrc[:, sl, 0],
                         start=False, stop=True, skip_group_check=True)
            mc = work.tile([128, CHUNK], F16, name="mc", tag="mc", bufs=3)
            S.activation(mc[:, :csz], zc[:, :csz], Act.Relu, bias=bee,
                         scale=1.0)
            if STOP_STAGE == "pipe1b" and layer == 0:
                return None
            if cc < NMAIN:
                evcc = work.tile([128, 2, CHUNK], F16, name="evcc", tag="evcc", bufs=3)
                ev_t, evm_t = evcc[:, 0, :], evcc[:, 1, :]
            else:
                sp = cc - NMAIN
                spsl = slice(CHUNK * sp, CHUNK * sp + csz)
                ev_t, evm_t = spev[:, 0, spsl], spev[:, 1, spsl]
            mc = mc[:, :csz]
            # L2 masked-src edges (hq[src] = -2e9): mc = 0 exactly, so they
            # add 0 to the numerator and exactly 1/32 to the denominator;
            # the denominator excess is removed analytically via `mm`
            # (adjacency matmul) instead of a per-edge mask multiply.
            S.activation(ev_t, mc, Act.Exp, bias=C["mln32_vec"], scale=tvec)
            V.tensor_tensor(out=evm_t, in0=ev_t, in1=mc, op=Alu.mult)
            if STOP_STAGE == "pipe1" and layer == 0:
                return None
            if cc < NMAIN:
                # windowed segment-sum via pairwise folds (f16 2x DVE mode)
                # + small f32 reduce: ~1550ns vs 2254 for one big 1x reduce
                f1 = work.tile([128, 2, 32, 16], F16, name="f1", tag="f1")
                ev4 = evcc[:, :, :].rearrange("p a (b c) -> p a b c", c=DEG)
                V.tensor_tensor(out=f1, in0=ev4[:, :, :, 0:16],
                                in1=ev4[:, :, :, 16:32], op=Alu.add)
                f2 = work.tile([128, 2, 32, 8], F16, name="f2", tag="f2")
                V.tensor_tensor(out=f2, in0=f1[:, :, :, 0:8],
                                in1=f1[:, :, :, 8:16], op=Alu.add)
                f3 = work.tile([128, 2, 32, 4], F16, name="f3", tag="f3")
                V.tensor_tensor(out=f3, in0=f2[:, :, :, 0:4],
                                in1=f2[:, :, :, 4:8], op=Alu.add)
                V.tensor_reduce(
                    out=sums[:, :, DEG * cc:DEG * (cc + 1)],
                    in_=f3, axis=mybir.AxisListType.X, op=Alu.add)
            if dbg_on and cc == 0:
                dbg("mc0", mc)
                dbg("ev0", ev_t)
                dbg("evm0", evm_t)
            if cc == NMAIN + 1:
                # spill scans + boundary gather (overlap with main chunks)
                Dsp = []
                for ti in range(2):
                    Ssp = nodep.tile([128, ESP + 1, 1], F32,
                                     name=f"Ssp{ti}_{layer}_{rep}",
                                     tag=f"Ssp{ti}")
                    V.memset(Ssp[:, 0:1, 0], 0.0)
                    V.tensor_tensor_scan(
                        out=Ssp[:, 1:, 0], data0=spev[:, ti, :],
                        data1=spev[:, ti, :], initial=0.0,
                        op0=Alu.add, op1=Alu.bypass)
                    gsp = nodep.tile([128, NEND, 1], F32,
                                     name=f"gsp{ti}_{layer}_{rep}",
                                     tag=f"gsp{ti}")
                    nc.gpsimd.ap_gather(
                        gsp, Ssp, C["spendidx"],
                        channels=128, num_elems=ESP + 1, d=1, num_idxs=NEND)
                    d = nodep.tile([128, N], F32, name=f"Dsp{ti}_{layer}_{rep}",
                                   tag=f"Dsp{ti}")
                    V.tensor_tensor(out=d, in0=gsp[:, 1:N + 1, 0],
                                    in1=gsp[:, 0:N, 0], op=Alu.subtract)
                    Dsp.append(d)
                if dbg_on:
                    dbg("spev0", spev[:, 0, :])
                    dbg("Ssp0", Ssp[:, :, 0])
                    dbg("Dsp0", Dsp[0])
                    dbg("Dsp1", Dsp[1])

        if dbg_on:
            dbg("sums0", sums[:, 0, :])
            dbg("sums1", sums[:, 1, :])
        return (sums, Dsp, evpad, evmpad)

    def aggr_half(E, hs, base_in, mm, layer, hi):
        """Softmax-mean for node slice hs; returns u = base + aggr + EPS."""
        sums, Dsp, evpad, evmpad = E
        HN = hs.stop - hs.start
        neg = []
        for ti, padv in ((0, evpad), (1, evmpad)):
            tot = nodep.tile([128, HN], F32, name=f"tot{ti}_{layer}{hi}_{rep}",
                             tag=f"tot{ti}")
            V.tensor_tensor(out=tot, in0=sums[:, ti, hs], in1=Dsp[ti][:, hs],
                            op=Alu.add)
            if ti == 0 and mm is not None:
                # tot += (mask-1)^T A / 32 (mm pre-scaled by 1/32)
                mmb = psume.tile([128, HN], F32, name="mmb", tag="e512")
                T.matmul(mmb, C["ones16b_stat"], mm[:, hs],
                         start=True, stop=True)
                tot2 = nodep.tile([128, HN], F32,
                                  name=f"tot2_{layer}{hi}_{rep}", tag="tot2")
                V.tensor_tensor(out=tot2, in0=mmb, in1=tot, op=Alu.add)
                tot = tot2
            ng = nodep.tile([128, HN], F32, name=f"ng{ti}_{layer}{hi}_{rep}",
                            tag=f"ng{ti}")
            V.scalar_tensor_tensor(out=ng, in0=C["padcnt"][:, hs], scalar=padv,
                                   in1=tot, op0=Alu.mult, op1=Alu.subtract)
            neg.append(ng)
        dm = nodep.tile([128, HN], F32, name=f"dm_{layer}{hi}_{rep}", tag="dm")
        V.tensor_scalar(dm, neg[0], -1e-16, None, Alu.min)
        rec = nodep.tile([128, HN], F32, name=f"rec_{layer}{hi}_{rep}",
                         tag="rec")
        V.reciprocal(rec, dm)
        ag = nodep.tile([128, HN], F32, name=f"ag_{layer}{hi}_{rep}", tag="ag")
        V.tensor_tensor(out=ag, in0=neg[1], in1=rec, op=Alu.mult)
        u = nodep.tile([128, HN], F32R, name=f"u_{layer}{hi}_{rep}", tag="u")
        V.scalar_tensor_tensor(out=u, in0=ag, scalar=EPS, in1=base_in[:, hs],
                               op0=Alu.add, op1=Alu.add)
        return u

    def mlp_half(uin, hs, wa_stat, ba_vec, gvec, bevec, wb_stat, layer, hi):
        HN = hs.stop - hs.start
        N2 = 2 * HN
        y1p = psumw.tile([128, 2, HN], F32, name=f"y1p_{layer}{hi}",
                         tag="wide")
        for half in range(2):
            T.matmul(y1p[:, half, :],
                     wa_stat[64 * half:64 * half + 64, :],
                     uin[64 * half:64 * half + 64, :],
                     start=True, stop=True)
        y1pf = y1p.rearrange("p a b -> p (a b)")
        y1s = nodep.tile([128, N2], F16, name=f"y1s_{layer}{hi}_{rep}",
                         tag="y1s")
        S.activation(y1s, y1pf, Act.Identity, bias=ba_vec, scale=1.0)
        sq = nodep.tile([128, N2], F16, name=f"sq_{layer}{hi}_{rep}", tag="sq")
        V.tensor_tensor(out=sq, in0=y1s, in1=y1s, op=Alu.mult)
        vp = psumw.tile([4, 2, HN], F32, name=f"vp_{layer}{hi}", tag="wide")
        for half in range(2):
            T.matmul(vp[:, half, :], C["ones32h_stat"],
                     sq[:, half * HN:half * HN + HN], start=True, stop=True)
        lnv = nodep.tile([4, N2], F32, name=f"lnv_{layer}{hi}_{rep}",
                         tag="st4", bufs=2)
        S.activation(lnv, vp.rearrange("p a b -> p (a b)"), Act.Ln,
                     bias=C["lneps_vec"], scale=1.0)
        rstd = nodep.tile([4, N2], F32R, name=f"rstd_{layer}{hi}_{rep}",
                          tag="st4", bufs=2)
        S.activation(rstd, lnv, Act.Exp, bias=0.0, scale=-0.5)
        rb = psumw.tile([128, 2, HN], F32, name=f"rb_{layer}{hi}", tag="wide")
        for half in range(2):
            T.matmul(rb[:, half, :], C["onesb32_stat"],
                     rstd[:, half * HN:half * HN + HN],
                     start=True, stop=True)
        vnorm = nodep.tile([128, N2], F32, name=f"vn_{layer}{hi}_{rep}",
                           tag="vn")
        V.tensor_tensor(out=vnorm, in0=y1s,
                        in1=rb.rearrange("p a b -> p (a b)"), op=Alu.mult)
        r1 = nodep.tile([128, N2], F16, name=f"r1_{layer}{hi}_{rep}", tag="r1")
        S.activation(r1, vnorm, Act.Relu, bias=bevec, scale=gvec)
        M = wb_stat.shape[1]
        outs = []
        for half in range(2):
            for q in range(2):
                yq = psum.tile([M, HN], F32, name=f"yq{half}{q}", tag="zc")
                T.matmul(yq, wb_stat[64 * q:64 * q + 64, :],
                         r1[64 * q:64 * q + 64, half * HN:half * HN + HN],
                         start=True, stop=True)
                outs.append(yq)
        return outs

    HALVES = (slice(0, N),)

    # ================= Layer 1 =================
    h1 = nodep.tile([128, N], F32, name=f"h1_{rep}", tag="h1")

    def half_pipe(layer, base_in, mm, wa, ba, gv, bev, wb, hout):
        def emit(E, hi, hs):
            u = aggr_half(E, hs, base_in, mm, layer, hi)
            y2q = mlp_half(u, hs, wa, ba, gv, bev, wb, layer, hi)
            for q in range(4):
                if layer == 0:
                    S.activation(hout[32 * q:32 * q + 32, hs], y2q[q],
                                 Act.Relu,
                                 bias=C["b1b_vec"][32 * q:32 * q + 32, :],
                                 scale=1.0)
                else:
                    sl, part = q // 2, q % 2
                    S.activation(hout[sl][64 * part:64 * part + 64, hs],
                                 y2q[q], Act.Relu,
                                 bias=C["b2b_vec"][64 * part:64 * part + 64, :],
                                 scale=1.0)
        return emit

    emit1 = half_pipe(0, hT[:, :N, 0], None, C["w1a_stat"], C["b1a_vec"],
                      C["g1_vec"], C["be1_vec"], C["w1bh_stat"], h1)
    E1 = edge_layer(0, hT, C["t1vec"], C["bee_vec"], dbg_on=True)
    if STOP_STAGE in ("pipe1", "pipe1a", "pipe1b"):
        return None
    emit1(E1, 0, HALVES[0])
    dbg("h1", h1)
    if STOP_STAGE == "mlp1":
        return stop_dma(h1[0:4, 0:G])

    # ================= score / topk mask / gates =================
    scp = psume.tile([8, N], F32, name="scp", tag="e512")
    T.matmul(scp, C["wpool_stat"], h1, start=True, stop=True)
    scs = nodep.tile([8, N], F32, name=f"scs_{rep}", tag="scs")
    S.activation(scs, scp, Act.Copy, bias=0.0, scale=1.0)
    dbg("score", scs)
    snode = nodep.tile([128, 4, 8], F32, name=f"snode_{rep}", tag="snode")
    for t in range(4):
        tp = psum1.tile([128, 8], F32, name="tp", tag="small")
        T.transpose(tp, scs[:, 128 * t:128 * (t + 1)], C["identT"][0:8, 0:8])
        S.activation(snode[:, t, :], tp, Act.Copy, bias=0.0, scale=1.0)
    sneg = nodep.tile([128, 4, 8], F32, name=f"sneg_{rep}", tag="sneg")
    V.tensor_scalar(sneg, snode, -1.0, None, Alu.mult)
    # rank: graphs 0-3 on Act (sign-sum), graphs 4-7 on DVE (is_gt count)
    rk = nodep.tile([128, 4, 8], F32, name=f"rk_{rep}", tag="rk")
    # interleave Act-half (g<4) and DVE-half (g>=4) so both engines run
    # concurrently despite the 2-deep sb psum ring
    for g in (0, 4, 1, 5, 2, 6, 3, 7):
        sb = psum.tile([128, N], F32, name="sb", tag="zc")
        T.matmul(sb, C["onesel_stat"][:, 128 * g:128 * (g + 1)],
                 scs, start=True, stop=True)
        if g < 4:
            for t in range(4):
                sga = work.tile([128, N], F16, name="sga", tag="sga")
                S.activation(sga, sb, Act.Sign, bias=sneg[:, t, g:g + 1],
                             scale=1.0, accum_out=rk[:, t, g:g + 1])
        else:
            for t in range(4):
                sgv = work.tile([128, N], F16, name="sgv", tag="sgv")
                V.tensor_scalar(sgv, sb, snode[:, t, g:g + 1], 0.0,
                                Alu.is_gt, Alu.add,
                                accum_out=rk[:, t, g:g + 1])
    dbg("rk", rk.rearrange("p a b -> p (a b)"))
    if STOP_STAGE == "rank":
        return stop_dma(rk[0:4, 0, 0:G])
    mask01 = nodep.tile([128, 4, 8], F32, name=f"mask01_{rep}", tag="mask01")
    V.tensor_scalar(mask01[:, :, 0:4], rk[:, :, 0:4], -1.0, None, Alu.is_le)
    V.tensor_scalar(mask01[:, :, 4:8], rk[:, :, 4:8], float(K) - 0.5, None,
                    Alu.is_le)
    dbg("mask", mask01.rearrange("p a b -> p (a b)"))
    ex = nodep.tile([128, 4, 8], F32, name=f"ex_{rep}", tag="ex")
    S.activation(ex, snode, Act.Exp, bias=0.0, scale=-2.0)
    exd = nodep.tile([128, 4, 8], F32, name=f"exd_{rep}", tag="exd")
    V.tensor_scalar(exd, ex, 1.0, None, Alu.add)
    exr = nodep.tile([128, 4, 8], F32, name=f"exr_{rep}", tag="exr")
    V.reciprocal(exr, exd)
    th = nodep.tile([128, 4, 8], F32, name=f"th_{rep}", tag="th")
    V.tensor_scalar(th, exr, 2.0, -1.0, Alu.mult, Alu.add)
    gate = nodep.tile([128, 4, 8], F32, name=f"gate_{rep}", tag="gate")
    V.tensor_tensor(out=gate, in0=th, in1=mask01, op=Alu.mult)
    gq = nodep.tile([128, 4, 8], F32, name=f"gq_{rep}", tag="gq")
    V.tensor_scalar(gq, mask01, -1.0, BIGNEG, Alu.add, Alu.mult)
    gqm = nodep.tile([72, N], F16, name=f"gqm_{rep}", tag="gqm")
    gfm, qfm, mfm = gqm[0:8, :], gqm[32:40, :], gqm[64:72, :]
    for t in range(4):
        tstack = work.tile([128, 72], F32, name="tstack", tag="tstack")
        V.memset(tstack[:, 8:32], 0.0)
        V.memset(tstack[:, 40:64], 0.0)
        V.tensor_copy(out=tstack[:, 0:8], in_=gate[:, t, :])
        V.tensor_copy(out=tstack[:, 32:40], in_=gq[:, t, :])
        V.tensor_copy(out=tstack[:, 64:72], in_=mask01[:, t, :])
        tq = psum1.tile([72, 128], F32, name="tq", tag="small")
        T.transpose(tq, tstack, C["identT"])
        S.activation(gqm[:, 128 * t:128 * (t + 1)], tq, Act.Copy,
                     bias=0.0, scale=1.0)
    # masked-src edge counts: mmg[g, n] = sum_m (mask_g[m]-1) * A_g[m, n];
    # per (g, block) lhsT is [128, 8] with only column g nonzero, so all 32
    # matmuls accumulate into one [8, N] psum (zero rows elsewhere)
    mfm1 = nodep.tile([8, N], F32, name=f"mfm1_{rep}", tag="mfm1")
    V.tensor_scalar(mfm1, mfm, -1.0, None, Alu.add)
    zsb = nodep.tile([128, 4, 64], F16, name=f"zsb_{rep}", tag="zsb")
    for b in range(4):
        zp = psum1.tile([128, 64], F32, name="zp", tag="small")
        T.matmul(zp, mfm1[:, 128 * b:128 * (b + 1)], C["d8_stat"],
                 start=True, stop=True)
        S.activation(zsb[:, b, :], zp, Act.Copy, bias=0.0, scale=1.0)
    mmg_ps = psumw.tile([8, N], F32, name="mmg_ps", tag="wide")
    for g in range(G):
        if g == 4:   # second half of A overwrites the buffer (WAR-tracked)
            nc.sync.dma_start(out=C["amat"], in_=C["amat_dram"][:, 16 * 512:])
        for b in range(4):
            T.matmul(mmg_ps, zsb[:, b, 8 * g:8 * g + 8],
                     C["amat"][:, ((g % 4) * 4 + b) * 512:
                               ((g % 4) * 4 + b + 1) * 512],
                     start=(g == 0 and b == 0), stop=(g == 7 and b == 3))
    mmg_s = nodep.tile([8, N], F16, name=f"mmg_s_{rep}", tag="mmg_s")
    S.activation(mmg_s, mmg_ps, Act.Copy, bias=0.0, scale=1.0 / DEG)

    gb = psume.tile([128, N], F32, name="gb", tag="e512")
    T.matmul(gb, C["ones16b_stat"], gfm, start=True, stop=True)
    hp = nodep.tile([128, N], F32, name=f"hp_{rep}", tag="hp")
    V.tensor_tensor(out=hp, in0=h1, in1=gb, op=Alu.mult)
    dbg("hp", hp)
    qb = psume.tile([128, N], F32, name="qb", tag="e512")
    T.matmul(qb, C["ones16b32_stat"][32:40, :], qfm,
             start=True, stop=True)
    hq = nodep.tile([128, NT, 1], F32, name=f"hq_{rep}", tag="hq")
    V.tensor_tensor(out=hq[:, :N, 0], in0=hp, in1=qb, op=Alu.add)
    S.activation(hq[:, N:, 0], qb[:, 0:NT - N], Act.Copy, bias=0.0, scale=0.0)
    dbg("hq", hq[:, :N, 0])
    if STOP_STAGE == "hq":
        return stop_dma(hq[0:4, 0:G, 0].bitcast(F32))

    # ================= Layer 2 =================
    h2 = [nodep.tile([128, N], F16, name=f"h2{sl}_{rep}", tag=f"h2{sl}")
          for sl in range(2)]
    emit2 = half_pipe(1, hp, mmg_s, C["w2a_stat"], C["b2a_vec"],
                      C["g2_vec"], C["be2_vec"], C["w2bh_stat"], h2)
    E2 = edge_layer(1, hq, C["t2vec"], C["bee_vec"], mm=mmg_s)
    if STOP_STAGE == "l2agg":
        return stop_dma(mmg_s[0:4, 0:G])
    emit2(E2, 0, HALVES[0])
    dbg("h2a", h2[0])
    dbg("h2b", h2[1])

    # ================= pooling + head =================
    pooled = []
    for sl, statname in ((0, "maskbc_statA"), (1, "maskbc_statB")):
        mb2 = psume.tile([128, N], F32, name=f"mbp{sl}", tag="e512")
        T.matmul(mb2, C[statname][64:72, :], mfm, start=True, stop=True)
        mbh = nodep.tile([128, N], F16, name=f"mbh{sl}_{rep}", tag=f"mbh{sl}")
        S.activation(mbh, mb2, Act.Copy, bias=0.0, scale=1.0)
        pl = nodep.tile([128, 1], F32, name=f"pl{sl}_{rep}", tag=f"pl{sl}")
        scratch = work.tile([128, N], F16, name="plscratch", tag="plscratch",
                            bufs=1)
        V.scalar_tensor_tensor(out=scratch, in0=h2[sl], scalar=1.0, in1=mbh,
                               op0=Alu.mult, op1=Alu.mult, accum_out=pl)
        pooled.append(pl)
    P8 = psum1.tile([32, G], F32, name="P8", tag="small")
    for g in range(G):
        sl, gg = g // 4, g % 4
        T.matmul(P8[:, g:g + 1],
                 C["selk_stat"][:, 32 * gg:32 * gg + 32],
                 pooled[sl], start=True, stop=True,
                 skip_group_check=True)
    p8s = nodep.tile([32, G], F32, name=f"p8s_{rep}", tag="p8s")
    S.activation(p8s, P8, Act.Copy, bias=0.0, scale=1.0)
    dbg("pooled", p8s)
    a1p = psume.tile([128, G], F32, name="a1p", tag="e512")
    T.matmul(a1p, C["wa_stat"], p8s, start=True, stop=True)
    a1 = nodep.tile([128, G], F32, name=f"a1_{rep}", tag="a1")
    S.activation(a1, a1p, Act.Relu, bias=C["ba_vec"], scale=1.0)
    op = psum1.tile([4, G], F32, name="op", tag="small")
    T.matmul(op, C["wo_stat"], a1, start=True, stop=True)
    oe = nodep.tile([4, G], F32, name=f"oe_{rep}", tag="oe")
    S.activation(oe, op, Act.Exp, bias=C["bo2_vec"], scale=-2.0)
    od = nodep.tile([4, G], F32, name=f"od_{rep}", tag="od")
    V.tensor_scalar(od, oe, 1.0, None, Alu.add)
    orr = nodep.tile([4, G], F32, name=f"orr_{rep}", tag="orr")
    V.reciprocal(orr, od)
    ot = nodep.tile([4, G], F32, name=f"ot_{rep}", tag="ot")
    V.tensor_scalar(ot, orr, 2.0, -1.0, Alu.mult, Alu.add)
    nc.sync.dma_start(out=out_ext, in_=ot)


# ----------------------------------------------------------------------------
# Self-contained entry point: kernel(**inputs) -> [64, 4] float32
# ----------------------------------------------------------------------------
import jax as _jax
from jax.sharding import Mesh as _Mesh, PartitionSpec as _PartitionSpec
from jax.experimental.shard_map import shard_map as _shard_map

_COMPILED = {}


def _build_and_jit():
    """Re-create the jitted executable on every call: re-executing a loaded
    NEFF leaves device state (semaphores) behind and corrupts the second run,
    so each kernel() invocation gets a fresh executable (BIR->NEFF is
    disk-cached, so this costs seconds, not a recompile)."""
    from concourse import bass2jax
    from concourse.bass2jax import _bass_exec_p, partition_id_tensor

    if "nc" in _COMPILED:
        nc = _COMPILED["nc"]
    else:
        nc = build_nc()
        _COMPILED["nc"] = nc
    bass2jax.install_neuronx_cc_hook()
    partition_name = (nc.partition_id_tensor.name
                      if nc.partition_id_tensor else None)
    in_names, out_names, out_avals, zero_outs = [], [], [], []
    for alloc in nc.m.functions[0].allocations:
        if not isinstance(alloc, mybir.MemoryLocationSet):
            continue
        nm = alloc.memorylocations[0].name
        if alloc.kind == "ExternalInput":
            if nm != partition_name:
                in_names.append(nm)
        elif alloc.kind == "ExternalOutput":
            out_names.append(nm)
            out_avals.append(_jax.core.ShapedArray(
                tuple(alloc.tensor_shape), mybir.dt.np(alloc.dtype)))
            zero_outs.append(np.zeros(tuple(alloc.tensor_shape),
                                      mybir.dt.np(alloc.dtype)))
    n_params = len(in_names)
    n_outs = len(out_avals)
    in_names_all = in_names + out_names
    if partition_name is not None:
        in_names_all.append(partition_name)
    donate = tuple(range(n_params, n_params + n_outs))

    def _body(*args):
        operands = list(args)
        if partition_name is not None:
            operands.append(partition_id_tensor())
        return tuple(_bass_exec_p.bind(
            *operands, out_avals=tuple(out_avals),
            in_names=tuple(in_names_all), out_names=tuple(out_names),
            lowering_input_output_aliases=(), sim_require_finite=True,
            sim_require_nnan=True, nc=nc))

    devices = _jax.devices()[:8]
    mesh = _Mesh(np.asarray(devices), ("core",))
    in_specs = (_PartitionSpec("core"),) * (n_params + n_outs)
    out_specs = (_PartitionSpec("core"),) * len(out_names)
    sharded = _jax.jit(
        _shard_map(_body, mesh=mesh, in_specs=in_specs, out_specs=out_specs,
                   check_rep=False),
        donate_argnums=donate, keep_unused=True)
    return (sharded, in_names, out_names, zero_outs)


def kernel(**inputs):
    """Full-input GNN forward on 8 TRN2 NeuronCores; returns [64, 4] f32."""
    sharded, in_names, out_names, zero_outs = _build_and_jit()
    core_maps = prep_inputs(inputs)
    concat_in = [np.concatenate([core_maps[c][nm] for c in range(8)], axis=0)
                 for nm in in_names]
    concat_zero = [np.zeros((8 * z.shape[0], *z.shape[1:]), z.dtype)
                   for z in zero_outs]
    out_arrs = sharded(*concat_in, *concat_zero)
    oi = out_names.index("out")
    full = np.asarray(out_arrs[oi]).reshape(8, 4, G)
    return np.concatenate([full[c].T for c in range(8)], axis=0)
